# revision 15
# baseline (speedup 1.0000x reference)
"""Trainium2 Bass kernel for nn_Base2D_80633716015315 (dense_mlp).

Math (per ray b, lobe d):
  feat = mlp0(cond)            # 10->32->32->32->32, relu between
  weight = relu(feat[:8]); v = feat[8:24] as [8,2]; ct = sigmoid(feat[24:32])
  v /= max(||v||, eps); wr = R(v) @ wi        # 2D rotation of wi by v
  x = [wr0, wr1, ct];  pe = [x, sin(2^k x), cos(2^k x)]  (27 feats)
  inter = relu(mlp(pe))        # 27->64->64->64->64->64, relu between + out
  lobes = softplus(inter . f1_w[d] + f1_b[d])
  pdf = sum_d lobes*weight
Returns (cos_theta [B,8], pdf [B]).

Sharding: pure data-parallel over B across 8 cores; weights replicated.

Per-core layout: Bc rows; partition index p = 8*g + d (g in 0..15 batch
groups, d in 0..7 lobes); free index j in 0..J-1; b = g*J + j.
Matmuls run feature-major (features on partitions) in float32r with
4-region tile_position packing; PSUM evacuation alternates ACT/DVE.
"""

import numpy as np

D = 8
G = 16
NCORES = 8

_BUILD_CACHE = {}
_RUN_KWARGS = {}
_LAST_RESULT = None


def _prep_consts(mlp0_ws, mlp0_bs, f0_ws, f0_bs, f1_w, f1_b):
    """Host-side constant tensors, all float32, laid out for the kernel."""
    f = lambda x: np.ascontiguousarray(np.asarray(x, dtype=np.float32))
    mlp0_ws = [f(w) for w in mlp0_ws]
    mlp0_bs = [f(b) for b in mlp0_bs]
    f0_ws = [f(w) for w in f0_ws]
    f0_bs = [f(b) for b in f0_bs]
    f1_w = f(f1_w)
    f1_b = f(f1_b)

    consts = {}
    # Permute the last mlp0 layer's output features so the blocks we need
    # later are contiguous: [v0(8), v1(8), wpre(8), ctpre(8)].
    perm = np.concatenate([
        np.arange(8, 24, 2),      # v0 = feat[8+2d]
        np.arange(9, 24, 2),      # v1 = feat[9+2d]
        np.arange(0, 8),          # weight pre-relu
        np.arange(24, 32),        # cos_theta pre-sigmoid
    ])
    w_last = mlp0_ws[3][perm, :]
    b_last = mlp0_bs[3][perm]

    # mlp0 weights: 4-block-diagonal lhsT (4 batch chunks per matmul).
    # Layer 0: [40,128] (K=4x10); layers 1-3: [128,128] (K=4x32).
    for l in range(4):
        w = w_last if l == 3 else mlp0_ws[l]
        b = b_last if l == 3 else mlp0_bs[l]
        K = w.shape[1]
        t = np.zeros((4 * K, 128), np.float32)
        for q in range(4):
            t[K * q:K * q + K, 32 * q:32 * q + 32] = w.T
        consts[f"w0_{l}"] = t
        consts[f"b0_{l}"] = np.tile(b, 4).reshape(128, 1).astype(np.float32)

    # f0 weights: 2-block-diagonal lhsT (2 group-streams per matmul).
    # Layer 0: [54,128] (K=2x27); layers 1-4: [128,128] (K=2x64).
    for l in range(5):
        w = f0_ws[l]
        K = w.shape[1]
        t = np.zeros((2 * K, 128), np.float32)
        t[0:K, 0:64] = w.T
        t[K:2 * K, 64:128] = w.T
        consts[f"wf_{l}"] = t
        consts[f"bf_{l}"] = np.tile(f0_bs[l], 2).reshape(128, 1).astype(np.float32)

    # f1: 16 lhsT variants [128, 32]; variant v serves pairs i with i%16==v
    # (the f1w column pattern has period 4 in i, so period 16 is safe).
    # Column c=2v+h gets f1w[(2i+h)%8] in row-block h; all other columns are
    # zero, so a 32-matmul accumulation group lands lobes for 16 pairs
    # densely in one [32, J] psum tile (row p%32).
    f1c = np.zeros((16 * 128, 32), np.float32)
    for v in range(16):
        for h in range(2):
            d = (2 * v + h) % 8
            f1c[128 * v + 64 * h:128 * v + 64 * h + 64, 2 * v + h] = f1_w.T[:, d]
    consts["sf1c"] = f1c
    # per-partition f1 bias for p = 8g+d -> f1_b[d]
    consts["bf1"] = np.tile(f1_b, G).reshape(128, 1).astype(np.float32)

    # pdf reduction selector: out[m] = sum_d PDFT[8m+d]
    sel = np.zeros((128, 16), np.float32)
    for p in range(128):
        sel[p, p // 8] = 1.0
    consts["sel"] = sel
    return consts


def _build(J, debug=False):
    """Build the single-core Bass program (run SPMD across all cores)."""
    from contextlib import ExitStack
    import concourse.bass as bass
    import concourse.mybir as mybir
    import concourse.tile as tile
    from concourse import bacc

    f32 = mybir.dt.float32
    f32r = mybir.dt.float32r
    AF = mybir.ActivationFunctionType
    ALU = mybir.AluOpType

    Bc = G * J
    N0 = J // 2
    NCH0 = Bc // N0          # 32 mlp0 chunks
    NPAIR = (G * D) // 2     # 64 stage-C pair iterations

    nc = bacc.Bacc("TRN2", target_bir_lowering=False, debug=debug)

    # ---- DRAM I/O ----
    wi_d = nc.dram_tensor("wi", [Bc, 2], f32, kind="ExternalInput")
    cond_d = nc.dram_tensor("cond", [Bc, 10], f32r, kind="ExternalInput")
    cdram = {}
    cshapes = {}
    for name, shape in [
        ("w0_0", (40, 128)),
        *[(f"w0_{l}", (128, 128)) for l in range(1, 4)],
        *[(f"b0_{l}", (128, 1)) for l in range(4)],
        ("wf_0", (54, 128)),
        *[(f"wf_{l}", (128, 128)) for l in range(1, 5)],
        *[(f"bf_{l}", (128, 1)) for l in range(5)],
        ("sf1c", (16 * 128, 32)), ("bf1", (128, 1)), ("sel", (128, 16)),
    ]:
        wdt = f32r if name[0] in "ws" else f32   # weights/sel f32r, biases f32
        cdram[name] = nc.dram_tensor(name, list(shape), wdt, kind="ExternalInput")
        cshapes[name] = shape
    ct_out_d = nc.dram_tensor("ct_out", [Bc, D], f32, kind="ExternalOutput")
    pdf_out_d = nc.dram_tensor("pdf_out", [Bc], f32, kind="ExternalOutput")

    # ACT/DVE duty split for psum evacuations (ACT also does sin/sigmoid etc)
    ACT_FRAC = 0.45
    evac_state = {"n": 0, "acts": 0}

    def pick_act():
        evac_state["n"] += 1
        want = int(evac_state["n"] * ACT_FRAC)
        if evac_state["acts"] < want:
            evac_state["acts"] += 1
            return True
        return False

    def evac(out_ap, in_ap, bias_ap, relu):
        if relu:
            if pick_act():
                nc.scalar.activation(out_ap, in_ap, AF.Relu, bias=bias_ap)
            else:
                nc.vector.tensor_scalar(out_ap, in_ap, bias_ap, 0.0,
                                        ALU.add, ALU.max)
        else:
            if pick_act():
                nc.scalar.activation(out_ap, in_ap, AF.Identity, bias=bias_ap)
            else:
                nc.vector.tensor_scalar(out_ap, in_ap, bias_ap, None, ALU.add)

    with tile.TileContext(nc) as tc, ExitStack() as ctx:
        pc = ctx.enter_context(tc.tile_pool(name="consts", bufs=1))
        pkeep = ctx.enter_context(tc.tile_pool(name="keep", bufs=1))
        pfeat = ctx.enter_context(tc.tile_pool(name="feat", bufs=1))
        pqall = ctx.enter_context(tc.tile_pool(name="qall", bufs=1))

        C = {}
        for name, shape in cshapes.items():
            if name == "sf1c":
                continue
            C[name] = pc.tile(list(shape), cdram[name].dtype, tag=name,
                              name=f"c_{name}")
            nc.sync.dma_start(out=C[name][:], in_=cdram[name].ap())
        F1C = []
        f1c_r = cdram["sf1c"].ap().rearrange("(v p) m -> v p m", v=16)
        for v in range(16):
            t = pc.tile([128, 32], f32r, tag=f"f1c{v}", name=f"f1c{v}")
            nc.sync.dma_start(out=t[:], in_=f1c_r[v])
            F1C.append(t)

        EPSB = pc.tile([128, 1], f32, tag="epsb", name="epsb")
        nc.vector.memset(EPSB[:], 1e-24)
        PIH = pc.tile([128, 1], f32, tag="pih", name="pih")
        nc.vector.memset(PIH[:], float(np.pi / 2))

        WPRE = pkeep.tile([128, J], f32, tag="wpre")    # -> WEIGHT (in place)
        LOB = pkeep.tile([128, J], f32r, tag="lob")      # lobespre -> lobes -> pdft
        PDFSB = pkeep.tile([16, J], f32, tag="pdfsb")

        FEAT = [pfeat.tile([128, J], f32, tag=f"feat{w}", name=f"feat{w}")
                for w in range(4)]
        QALL = pqall.tile([128, 27 * J], f32r, tag="qall")

        pps = ctx.enter_context(tc.tile_pool(name="ps", bufs=3, space="PSUM"))
        ppsf1 = ctx.enter_context(tc.tile_pool(name="psf1", bufs=1, space="PSUM"))

        # ================= Stage A: mlp0 (4-block-diagonal) =================
        with tc.tile_pool(name="mlp0io", bufs=1) as pm, \
             tc.tile_pool(name="h0", bufs=2) as ph0:
            CONDT = pm.tile([40, 8 * N0], f32r, tag="condt", name="condt")
            cond_r = cond_d.ap().rearrange("(q b) f -> q f b", q=4)
            for q in range(4):
                nc.sync.dma_start(out=CONDT[10 * q:10 * q + 10, :], in_=cond_r[q])

            KS0 = [40, 128, 128, 128]
            for jp in range(4):                 # 4 pairs of chunk-waves
                h_prev = None
                for l in range(4):
                    ps = pps.tile([128, J], f32, tag="ps", name="ps")
                    K = KS0[l]
                    for c in range(2):
                        if l == 0:
                            rhs = CONDT[:, (2 * jp + c) * N0:(2 * jp + c + 1) * N0]
                        else:
                            rhs = h_prev[:, c * N0:(c + 1) * N0]
                        nc.tensor.matmul(
                            ps[:, c * N0:(c + 1) * N0],
                            C[f"w0_{l}"][0:K, :],
                            rhs,
                        )
                    if l < 3:
                        h = ph0.tile([128, J], f32r, tag="h0", name="h0t")
                        evac(h[:], ps[:], C[f"b0_{l}"][:], relu=True)
                        h_prev = h
                    else:
                        evac(FEAT[jp][:], ps[:], C[f"b0_{l}"][:], relu=False)

        # ============ V-shuffle + Stage B (elementwise) ============
        with tc.tile_pool(name="bvars", bufs=1) as pb:
            V0 = pb.tile([128, J], f32, tag="v0")
            V1 = pb.tile([128, J], f32, tag="v1")
            CTP = pb.tile([128, J], f32, tag="ctp")
            WIX = pb.tile([128, J], f32, tag="wix")
            WIY = pb.tile([128, J], f32, tag="wiy")
            T0 = pb.tile([128, J], f32, tag="t0")
            T1 = pb.tile([128, J], f32, tag="t1")

            for n in range(NCH0):
                # chunk n = b*8 + jw lives in FEAT[jw//2] rows 32b, cols (jw%2)
                b_blk, jw = n // 8, n % 8
                jp, c = jw // 2, jw % 2
                g, jh = n // 2, n % 2
                for dst, base in ((V0, 0), (V1, 8), (WPRE, 16), (CTP, 24)):
                    nc.sync.dma_start(
                        out=dst[8 * g:8 * g + 8, jh * N0:(jh + 1) * N0],
                        in_=FEAT[jp][32 * b_blk + base:32 * b_blk + base + 8,
                                     c * N0:(c + 1) * N0])

            # wi broadcast: WIX[p=8g+d, j] = wi[g*J+j, 0]
            for g in range(G):
                for comp, dst in ((0, WIX), (1, WIY)):
                    src = bass.AP(tensor=wi_d.ap().tensor,
                                  offset=2 * g * J + comp,
                                  ap=[[0, 8], [2, J]])
                    nc.sync.dma_start(out=dst[8 * g:8 * g + 8, :], in_=src)

            wr0 = QALL[:, 0:J]
            wr1 = QALL[:, J:2 * J]
            ct = QALL[:, 2 * J:3 * J]

            # rinv = exp(-0.5*ln(v0^2+v1^2+eps))
            nc.vector.tensor_tensor(T0[:], V0[:], V0[:], ALU.mult)
            nc.vector.tensor_tensor(T1[:], V1[:], V1[:], ALU.mult)
            nc.vector.tensor_tensor(T0[:], T0[:], T1[:], ALU.add)
            nc.scalar.activation(T1[:], T0[:], AF.Ln, bias=EPSB[:])
            nc.scalar.activation(T0[:], T1[:], AF.Exp, scale=-0.5)
            # normalize v
            nc.vector.tensor_tensor(V0[:], V0[:], T0[:], ALU.mult)
            nc.vector.tensor_tensor(V1[:], V1[:], T0[:], ALU.mult)
            # wr = R(v) wi
            nc.vector.tensor_tensor(T0[:], V0[:], WIX[:], ALU.mult)
            nc.vector.tensor_tensor(T1[:], V1[:], WIY[:], ALU.mult)
            nc.vector.tensor_tensor(wr0, T0[:], T1[:], ALU.subtract)
            nc.vector.tensor_tensor(T0[:], V1[:], WIX[:], ALU.mult)
            nc.vector.tensor_tensor(T1[:], V0[:], WIY[:], ALU.mult)
            nc.vector.tensor_tensor(wr1, T0[:], T1[:], ALU.add)
            # ct = sigmoid(ctpre)
            nc.scalar.activation(ct, CTP[:], AF.Sigmoid)
            # weight = relu(wpre), in place
            nc.vector.tensor_scalar_max(WPRE[:], WPRE[:], 0.0)
            # positional encoding. ACT Sin only accepts [-pi, pi]; k=0
            # inputs are in range (|wr|<=1, ct in [0,1], +pi/2 <= 2.58), so
            # compute sin/cos at k=0 on ACT and k=1..3 by double-angle
            # recurrences (sin2x=2 s c, cos2x=1-2 s^2) on DVE/GPSIMD.
            T2 = pb.tile([128, J], f32, tag="t2", name="t2")
            T3 = pb.tile([128, J], f32, tag="t3", name="t3")

            def sinf(k, ci):
                f = 3 + 3 * k + ci
                return QALL[:, f * J:(f + 1) * J]

            def cosf(k, ci):
                f = 15 + 3 * k + ci
                return QALL[:, f * J:(f + 1) * J]

            for ci, comp in enumerate((wr0, wr1, ct)):
                nc.scalar.activation(sinf(0, ci), comp, AF.Sin)
                nc.scalar.activation(cosf(0, ci), comp, AF.Sin, bias=PIH[:])
            for ci in range(3):
                eng = nc.gpsimd if ci == 1 else nc.vector
                ta, tb = (T2, T3) if ci == 1 else (T0, T1)
                for k in range(3):
                    s_k, c_k = sinf(k, ci), cosf(k, ci)
                    eng.tensor_tensor(ta[:], s_k, c_k, ALU.mult)
                    eng.tensor_scalar(sinf(k + 1, ci), ta[:], 2.0, None, ALU.mult)
                    eng.tensor_tensor(tb[:], s_k, s_k, ALU.mult)
                    eng.tensor_scalar(cosf(k + 1, ci), tb[:], -2.0, 1.0,
                                      ALU.mult, ALU.add)

            # cos_theta output (strided DMA out of the QALL ct slice)
            ct_r = ct_out_d.ap().rearrange("(g j) d -> g d j", g=G)
            for g in range(G):
                nc.sync.dma_start(out=ct_r[g],
                                  in_=QALL[8 * g:8 * g + 8, 2 * J:3 * J]
                                  .bitcast(f32))

        # ================= Stage C: f0 MLP + f1 =================
        # Per pair i: streams A=p0 (feature rows 0-63) and B=p1 (64-127),
        # block-diagonal lhsT, full-array matmuls, no tile_position.
        KSF = [54, 128, 128, 128, 128]
        with tc.tile_pool(name="pex", bufs=3) as ppex, \
             tc.tile_pool(name="hc", bufs=6) as phc, \
             tc.tile_pool(name="scrap", bufs=2) as pscrap:
            psf1 = None
            for i in range(NPAIR):
                p0, p1 = 2 * i, 2 * i + 1
                pex = ppex.tile([54, J], f32r, tag="pex", name="pex")
                for h, p in ((0, p0), (1, p1)):
                    src = QALL[p:p + 1, :].rearrange("p (f j) -> p f j", f=27)
                    nc.sync.dma_start(out=pex[27 * h:27 * h + 27, :], in_=src)

                h_prev = pex
                for l in range(5):
                    K = KSF[l]
                    ps = pps.tile([128, J], f32, tag="ps", name="ps")
                    for j in range(2):
                        nc.tensor.matmul(
                            ps[:, j * N0:(j + 1) * N0],
                            C[f"wf_{l}"][0:K, :],
                            h_prev[0:K, j * N0:(j + 1) * N0],
                        )
                    hn = phc.tile([128, J], f32r, tag="h", name="hct")
                    evac(hn[:], ps[:], C[f"bf_{l}"][:], relu=True)
                    h_prev = hn

                # f1: 16-pair accumulation group -> lobes land densely at
                # psum row p%32 (see _prep_consts for the F1C layout).
                if i % 16 == 0:
                    psf1 = ppsf1.tile([32, J], f32, tag="psf1", name="psf1")
                for j in range(2):
                    nc.tensor.matmul(
                        psf1[:, j * N0:(j + 1) * N0],
                        F1C[i % 16][:, :],
                        h_prev[:, j * N0:(j + 1) * N0],
                        start=(i % 16 == 0),
                        stop=(i % 16 == 15),
                        skip_group_check=True,
                    )
                if i % 16 == 15:
                    e = i // 16
                    scrap = pscrap.tile([32, J], f32r, tag="scrap", name="scrap")
                    if pick_act():
                        nc.scalar.activation(scrap[:], psf1[:], AF.Identity)
                    else:
                        nc.vector.tensor_copy(scrap[:], psf1[:])
                    nc.sync.dma_start(out=LOB[32 * e:32 * e + 32, :],
                                      in_=scrap[:])

        # ================= Final: softplus + pdf =================
        # X = lobespre + f1_b; softplus(X) = max(X,0) + ln(1 + exp(-|X|))
        # (no Softplus activation table on this build; FEAT tiles are dead
        # here and reused as scratch)
        S0, S1 = FEAT[0], FEAT[1]
        nc.vector.tensor_scalar(LOB[:], LOB[:], C["bf1"][:], None, ALU.add)
        nc.vector.tensor_scalar(S0[:], LOB[:], -1.0, None, ALU.mult)
        nc.vector.tensor_tensor(S0[:], LOB[:], S0[:], ALU.min)
        nc.scalar.activation(S0[:], S0[:], AF.Exp)
        nc.scalar.activation(S0[:], S0[:], AF.Ln, bias=1.0)
        nc.vector.tensor_scalar_max(S1[:], LOB[:], 0.0)
        nc.vector.tensor_tensor(LOB[:], S0[:], S1[:], ALU.add)
        nc.vector.tensor_tensor(LOB[:], LOB[:], WPRE[:], ALU.mult)
        pspdf = pps.tile([128, J], f32, tag="ps", name="pspdf")
        for j in range(2):
            nc.tensor.matmul(
                pspdf[0:16, j * N0:(j + 1) * N0],
                C["sel"][:, :],
                LOB[:, j * N0:(j + 1) * N0],
            )
        nc.scalar.activation(PDFSB[:], pspdf[0:16, :], AF.Identity)
        pdf_r = pdf_out_d.ap().rearrange("(g j) -> g j", g=G)
        nc.sync.dma_start(out=pdf_r, in_=PDFSB[:])

    nc.compile()
    return nc


def _get_nc(J, debug=False):
    key = (J, debug)
    if key not in _BUILD_CACHE:
        _BUILD_CACHE[key] = _build(J, debug=debug)
    return _BUILD_CACHE[key]


def kernel(wi, cond, mlp0_ws, mlp0_bs, f0_ws, f0_bs, f1_w, f1_b):
    from concourse.bass_utils import run_bass_kernel_spmd

    wi = np.ascontiguousarray(np.asarray(wi, dtype=np.float32))
    cond = np.ascontiguousarray(np.asarray(cond, dtype=np.float32))
    B = wi.shape[0]
    Bc = B // NCORES
    J = Bc // G
    consts = _prep_consts(mlp0_ws, mlp0_bs, f0_ws, f0_bs, f1_w, f1_b)

    nc = _get_nc(J)
    in_maps = []
    for k in range(NCORES):
        m = {"wi": wi[k * Bc:(k + 1) * Bc], "cond": cond[k * Bc:(k + 1) * Bc]}
        m.update(consts)
        in_maps.append(m)
    global _LAST_RESULT
    res = run_bass_kernel_spmd(nc, in_maps, core_ids=list(range(NCORES)),
                               **_RUN_KWARGS)
    _LAST_RESULT = res
    cos_theta = np.concatenate([r["ct_out"] for r in res.results], axis=0)
    pdf = np.concatenate([r["pdf_out"] for r in res.results], axis=0)
    return cos_theta, pdf


# revision 30
# speedup vs baseline: 1.0596x; 1.0596x over previous
"""Trainium2 Bass kernel for nn_Base2D_80633716015315 (dense_mlp).

Math (per ray b, lobe d):
  feat = mlp0(cond)            # 10->32->32->32->32, relu between
  weight = relu(feat[:8]); v = feat[8:24] as [8,2]; ct = sigmoid(feat[24:32])
  v /= max(||v||, eps); wr = R(v) @ wi        # 2D rotation of wi by v
  x = [wr0, wr1, ct];  pe = [x, sin(2^k x), cos(2^k x)]  (27 feats)
  inter = relu(mlp(pe))        # 27->64->64->64->64->64, relu between + out
  lobes = softplus(inter . f1_w[d] + f1_b[d])
  pdf = sum_d lobes*weight
Returns (cos_theta [B,8], pdf [B]).

Sharding: pure data-parallel over B across 8 cores; weights replicated.

Per-core layout: Bc rows; partition index p = 8*g + d (g in 0..15 batch
groups, d in 0..7 lobes); free index j in 0..J-1; b = g*J + j.
Matmuls run feature-major (features on partitions) in float32r with
4-region tile_position packing; PSUM evacuation alternates ACT/DVE.
"""

import numpy as np

D = 8
G = 16
NCORES = 8

_BUILD_CACHE = {}
_RUN_KWARGS = {}
_LAST_RESULT = None


def _prep_consts(mlp0_ws, mlp0_bs, f0_ws, f0_bs, f1_w, f1_b):
    """Host-side constant tensors, all float32, laid out for the kernel."""
    f = lambda x: np.ascontiguousarray(np.asarray(x, dtype=np.float32))
    mlp0_ws = [f(w) for w in mlp0_ws]
    mlp0_bs = [f(b) for b in mlp0_bs]
    f0_ws = [f(w) for w in f0_ws]
    f0_bs = [f(b) for b in f0_bs]
    f1_w = f(f1_w)
    f1_b = f(f1_b)

    consts = {}
    # Permute the last mlp0 layer's output features so the blocks we need
    # later are contiguous: [v0(8), v1(8), wpre(8), ctpre(8)].
    perm = np.concatenate([
        np.arange(8, 24, 2),      # v0 = feat[8+2d]
        np.arange(9, 24, 2),      # v1 = feat[9+2d]
        np.arange(0, 8),          # weight pre-relu
        np.arange(24, 32),        # cos_theta pre-sigmoid
    ])
    w_last = mlp0_ws[3][perm, :]
    b_last = mlp0_bs[3][perm]

    # One packed lhsT bundle [9, 128, 128]:
    #  slots 0-3: mlp0 4-block-diagonal (layer0 K=40, else K=128)
    #  slots 4-8: f0 2-block-diagonal (layer0 K=54, else K=128)
    wall = np.zeros((9, 128, 128), np.float32)
    for l in range(4):
        w = w_last if l == 3 else mlp0_ws[l]
        K = w.shape[1]
        for q in range(4):
            wall[l, K * q:K * q + K, 32 * q:32 * q + 32] = w.T
    for l in range(5):
        w = f0_ws[l]
        K = w.shape[1]
        wall[4 + l, 0:K, 0:64] = w.T
        wall[4 + l, K:2 * K, 64:128] = w.T
    # layer-0 f0 block replicated at row 64 too (odd pairs read their PEX
    # block at base partition 64; matmul requires lhsT/rhs bases to match)
    w = f0_ws[0]
    wall[4, 64:64 + 27, 0:64] = w.T
    wall[4, 64 + 27:64 + 54, 64:128] = w.T
    consts["swall"] = wall.reshape(9 * 128, 128)

    # Bias bundle [128, 12]: cols 0-3 mlp0 (x4 repl), 4-8 f0 (x2 repl),
    # 9 f1 (f1_b[p%8]), 10 eps, 11 pi/2.
    ball = np.zeros((128, 12), np.float32)
    for l in range(4):
        b = b_last if l == 3 else mlp0_bs[l]
        ball[:, l] = np.tile(b, 4)
    for l in range(5):
        ball[:, 4 + l] = np.tile(f0_bs[l], 2)
    ball[:, 9] = np.tile(f1_b, G)
    ball[:, 10] = 1e-24
    ball[:, 11] = np.pi / 2
    consts["ball"] = ball

    # f1: 16 lhsT variants [128, 32]; variant v serves pairs i with i%16==v.
    # Column c=2v+h gets f1w[(2i+h)%8] in row-block h; other columns zero, so
    # a 32-matmul accumulation group lands lobes for 16 pairs densely in one
    # [32, J] psum tile (row p%32).
    f1c = np.zeros((16 * 128, 32), np.float32)
    for v in range(16):
        for h in range(2):
            d = (2 * v + h) % 8
            f1c[128 * v + 64 * h:128 * v + 64 * h + 64, 2 * v + h] = f1_w.T[:, d]
    consts["sf1c"] = f1c

    # pdf reduction selector: out[m] = sum_d PDFT[8m+d]
    sel = np.zeros((128, 16), np.float32)
    for p in range(128):
        sel[p, p // 8] = 1.0
    consts["sel"] = sel
    return consts


def _build(J, debug=False):
    """Build the single-core Bass program (run SPMD across all cores)."""
    from contextlib import ExitStack
    import concourse.bass as bass
    import concourse.mybir as mybir
    import concourse.tile as tile
    from concourse import bacc

    f32 = mybir.dt.float32
    f32r = mybir.dt.float32r
    AF = mybir.ActivationFunctionType
    ALU = mybir.AluOpType

    Bc = G * J
    N0 = J // 2
    NCH0 = Bc // N0          # 32 mlp0 chunks
    NPAIR = (G * D) // 2     # 64 stage-C pair iterations

    nc = bacc.Bacc("TRN2", target_bir_lowering=False, debug=debug)

    # ---- DRAM I/O ----
    wi_d = nc.dram_tensor("wi", [Bc, 2], f32, kind="ExternalInput")
    cond_d = nc.dram_tensor("cond", [Bc, 10], f32r, kind="ExternalInput")
    cdram = {}
    cshapes = {}
    for name, shape in [
        ("swall", (9 * 128, 128)),
        ("ball", (128, 12)),
        ("sf1c", (16 * 128, 32)),
        ("sel", (128, 16)),
    ]:
        wdt = f32r if name[0] == "s" else f32   # lhsT bundles f32r
        cdram[name] = nc.dram_tensor(name, list(shape), wdt, kind="ExternalInput")
        cshapes[name] = shape
    ct_out_d = nc.dram_tensor("ct_out", [Bc, D], f32, kind="ExternalOutput")
    pdf_out_d = nc.dram_tensor("pdf_out", [Bc], f32, kind="ExternalOutput")

    # ACT/DVE duty split for psum evacuations (ACT also does sin/sigmoid etc)
    ACT_FRAC = 0.55
    evac_state = {"n": 0, "acts": 0}

    def pick_act():
        evac_state["n"] += 1
        want = int(evac_state["n"] * ACT_FRAC)
        if evac_state["acts"] < want:
            evac_state["acts"] += 1
            return True
        return False

    def evac(out_ap, in_ap, bias_ap, relu):
        if relu:
            if pick_act():
                nc.scalar.activation(out_ap, in_ap, AF.Relu, bias=bias_ap)
            else:
                nc.vector.tensor_scalar(out_ap, in_ap, bias_ap, 0.0,
                                        ALU.add, ALU.max)
        else:
            if pick_act():
                nc.scalar.activation(out_ap, in_ap, AF.Identity, bias=bias_ap)
            else:
                nc.vector.tensor_scalar(out_ap, in_ap, bias_ap, None, ALU.add)

    with tile.TileContext(nc) as tc, ExitStack() as ctx:
        pc = ctx.enter_context(tc.tile_pool(name="consts", bufs=1))
        pkeep = ctx.enter_context(tc.tile_pool(name="keep", bufs=1))
        pfeat = ctx.enter_context(tc.tile_pool(name="feat", bufs=1))
        pqall = ctx.enter_context(tc.tile_pool(name="qall", bufs=1))

        WALL = pc.tile([128, 9 * 128], f32r, tag="wall", name="wall")
        nc.sync.dma_start(
            out=WALL[:],
            in_=bass.AP(tensor=cdram["swall"].ap().tensor, offset=0,
                        ap=[[128, 128], [128 * 128, 9], [1, 128]]))
        BALL = pc.tile([128, 12], f32, tag="ball", name="ball")
        nc.sync.dma_start(out=BALL[:], in_=cdram["ball"].ap())
        F1CALL = pc.tile([128, 16 * 32], f32r, tag="f1call", name="f1call")
        nc.sync.dma_start(
            out=F1CALL[:],
            in_=bass.AP(tensor=cdram["sf1c"].ap().tensor, offset=0,
                        ap=[[32, 128], [128 * 32, 16], [1, 32]]))
        SEL = pc.tile([128, 16], f32r, tag="sel", name="sel")
        nc.sync.dma_start(out=SEL[:], in_=cdram["sel"].ap())

        def wmat(slot, K):
            return WALL[0:K, 128 * slot:128 * slot + 128]

        def bias(col):
            return BALL[:, col:col + 1]

        WPRE = pkeep.tile([128, J], f32, tag="wpre")    # -> WEIGHT (in place)
        LOB = pkeep.tile([128, J], f32r, tag="lob")      # lobespre -> lobes -> pdft
        PDFSB = pkeep.tile([16, J], f32, tag="pdfsb")

        FEAT = pfeat.tile([128, 4 * J], f32, tag="feat", name="feat")
        QALL = pqall.tile([128, 27 * J], f32r, tag="qall")

        pps = ctx.enter_context(tc.tile_pool(name="ps", bufs=3, space="PSUM"))
        ppsf1 = ctx.enter_context(tc.tile_pool(name="psf1", bufs=1, space="PSUM"))

        # ================= Stage A: mlp0 (4-block-diagonal) =================
        with tc.tile_pool(name="mlp0io", bufs=1) as pm, \
             tc.tile_pool(name="h0", bufs=2) as ph0:
            CONDT = pm.tile([40, 8 * N0], f32r, tag="condt", name="condt")
            cond_r = cond_d.ap().rearrange("(q b) f -> q f b", q=4)
            for q in range(4):
                nc.sync.dma_start(out=CONDT[10 * q:10 * q + 10, :],
                                  in_=cond_r[q])

            KS0 = [40, 128, 128, 128]
            for jp in range(4):                 # 4 pairs of chunk-waves
                h_prev = None
                for l in range(4):
                    ps = pps.tile([128, J], f32, tag="ps", name="ps")
                    K = KS0[l]
                    for c in range(2):
                        if l == 0:
                            rhs = CONDT[:, (2 * jp + c) * N0:(2 * jp + c + 1) * N0]
                        else:
                            rhs = h_prev[:, c * N0:(c + 1) * N0]
                        nc.tensor.matmul(
                            ps[:, c * N0:(c + 1) * N0],
                            wmat(l, K),
                            rhs,
                        )
                    if l < 3:
                        h = ph0.tile([128, J], f32r, tag="h0", name="h0t")
                        evac(h[:], ps[:], bias(l), relu=True)
                        h_prev = h
                    else:
                        evac(FEAT[:, jp * J:(jp + 1) * J], ps[:], bias(l),
                             relu=False)

        # ============ V-shuffle + Stage B (elementwise) ============
        with tc.tile_pool(name="bvars", bufs=1) as pb:
            V0 = pb.tile([128, J], f32, tag="v0")
            V1 = pb.tile([128, J], f32, tag="v1")
            CTP = pb.tile([128, J], f32, tag="ctp")
            WIXY = pb.tile([128, 2 * J], f32, tag="wixy", name="wixy")
            T0 = pb.tile([128, J], f32, tag="t0")
            T1 = pb.tile([128, J], f32, tag="t1")

            # V-shuffle: direct SBUF->SBUF, one DMA per (quantity, g):
            # chunks (2g, 2g+1) both live in FEAT[:, jp*J:(jp+1)*J] rows
            # 32b+8*qty (b=g//4, jp=g%4) covering the full J columns.
            for qty, dst in ((0, V0), (1, V1), (2, WPRE), (3, CTP)):
                for g in range(G):
                    b_blk, jp = g // 4, g % 4
                    nc.sync.dma_start(
                        out=dst[8 * g:8 * g + 8, :],
                        in_=FEAT[32 * b_blk + 8 * qty:32 * b_blk + 8 * qty + 8,
                                 jp * J:(jp + 1) * J])

            # wi broadcast: WIXY[p=8g+d, comp*J+j] = wi[g*J+j, comp]
            for comp in range(2):
                for g in range(G):
                    src = bass.AP(tensor=wi_d.ap().tensor,
                                  offset=2 * g * J + comp,
                                  ap=[[0, 8], [2, J]])
                    nc.sync.dma_start(
                        out=WIXY[8 * g:8 * g + 8, comp * J:(comp + 1) * J],
                        in_=src)
            WIX = WIXY[:, 0:J]
            WIY = WIXY[:, J:2 * J]

            wr0 = QALL[:, 0:J]
            wr1 = QALL[:, J:2 * J]
            ct = QALL[:, 2 * J:3 * J]

            # rinv = exp(-0.5*ln(v0^2+v1^2+eps))
            nc.vector.tensor_tensor(T0[:], V0[:], V0[:], ALU.mult)
            nc.vector.tensor_tensor(T1[:], V1[:], V1[:], ALU.mult)
            nc.vector.tensor_tensor(T0[:], T0[:], T1[:], ALU.add)
            nc.scalar.activation(T1[:], T0[:], AF.Ln, bias=bias(10))
            nc.scalar.activation(T0[:], T1[:], AF.Exp, scale=-0.5)
            # normalize v
            nc.vector.tensor_tensor(V0[:], V0[:], T0[:], ALU.mult)
            nc.vector.tensor_tensor(V1[:], V1[:], T0[:], ALU.mult)
            # wr = R(v) wi
            nc.vector.tensor_tensor(T0[:], V0[:], WIX, ALU.mult)
            nc.vector.tensor_tensor(T1[:], V1[:], WIY, ALU.mult)
            nc.vector.tensor_tensor(wr0, T0[:], T1[:], ALU.subtract)
            nc.vector.tensor_tensor(T0[:], V1[:], WIX, ALU.mult)
            nc.vector.tensor_tensor(T1[:], V0[:], WIY, ALU.mult)
            nc.vector.tensor_tensor(wr1, T0[:], T1[:], ALU.add)
            # ct = sigmoid(ctpre)
            nc.scalar.activation(ct, CTP[:], AF.Sigmoid)
            # weight = relu(wpre), in place
            nc.vector.tensor_scalar_max(WPRE[:], WPRE[:], 0.0)
            # positional encoding. ACT Sin only accepts [-pi, pi]; k=0
            # inputs are in range (|wr|<=1, ct in [0,1], +pi/2 <= 2.58), so
            # compute sin/cos at k=0 on ACT and k=1..3 by double-angle
            # recurrences (sin2x=2 s c, cos2x=1-2 s^2) on DVE/GPSIMD.
            T2 = pb.tile([128, J], f32, tag="t2", name="t2")
            T3 = pb.tile([128, J], f32, tag="t3", name="t3")

            def sinf(k, ci):
                f = 3 + 3 * k + ci
                return QALL[:, f * J:(f + 1) * J]

            def cosf(k, ci):
                f = 15 + 3 * k + ci
                return QALL[:, f * J:(f + 1) * J]

            for ci, comp in enumerate((wr0, wr1, ct)):
                nc.scalar.activation(sinf(0, ci), comp, AF.Sin)
                nc.scalar.activation(cosf(0, ci), comp, AF.Sin, bias=bias(11))
            for ci in range(3):
                eng = nc.gpsimd if ci == 1 else nc.vector
                ta, tb = (T2, T3) if ci == 1 else (T0, T1)
                for k in range(3):
                    s_k, c_k = sinf(k, ci), cosf(k, ci)
                    eng.tensor_tensor(ta[:], s_k, c_k, ALU.mult)
                    nc.vector.tensor_scalar(sinf(k + 1, ci), ta[:], 2.0,
                                            None, ALU.mult)
                    eng.tensor_tensor(tb[:], s_k, s_k, ALU.mult)
                    nc.vector.tensor_scalar(cosf(k + 1, ci), tb[:], -2.0,
                                            1.0, ALU.mult, ALU.add)

            # cos_theta output: per-g strided DMAs straight to HBM
            for g in range(G):
                ctdst = bass.AP(tensor=ct_out_d.ap().tensor, offset=g * J * 8,
                                ap=[[1, 8], [8, J]])
                nc.scalar.dma_start(
                    out=ctdst,
                    in_=QALL[8 * g:8 * g + 8, 2 * J:3 * J].bitcast(f32))

        # ================= Stage C: f0 MLP + f1 =================
        # Per pair i: streams A=p0 (feature rows 0-63) and B=p1 (64-127),
        # block-diagonal lhsT, full-array matmuls, no tile_position. PEX
        # tiles hold 2 pairs (4 p's, [108, J]) loaded by one DMA each.
        KSF = [54, 128, 128, 128, 128]
        with tc.tile_pool(name="pex", bufs=3) as ppex, \
             tc.tile_pool(name="hc", bufs=6) as phc, \
             tc.tile_pool(name="scrap", bufs=2) as pscrap:
            psf1 = None
            pex2 = None
            qpitch = QALL.ap[0][0]
            for i in range(NPAIR):
                if i % 2 == 0:
                    pex2 = ppex.tile([128, J], f32r, tag="pex", name="pex")
                    for half in range(2):
                        src = bass.AP(
                            tensor=QALL.tensor,
                            offset=QALL.offset + (2 * i + 2 * half) * qpitch,
                            ap=[[qpitch, 2], [J, 27], [1, J]])
                        nc.sync.dma_start(
                            out=pex2[64 * half:64 * half + 54, :], in_=src)
                pex = pex2[64 * (i % 2):64 * (i % 2) + 54, :]

                h_prev = pex
                base0 = 64 * (i % 2)
                for l in range(5):
                    K = KSF[l]
                    base = base0 if l == 0 else 0
                    ps = pps.tile([128, J], f32, tag="ps", name="ps")
                    for j in range(2):
                        nc.tensor.matmul(
                            ps[:, j * N0:(j + 1) * N0],
                            WALL[base:base + K,
                                 128 * (4 + l):128 * (4 + l) + 128],
                            h_prev[0:K, j * N0:(j + 1) * N0]
                            if l > 0 else
                            pex2[base:base + K, j * N0:(j + 1) * N0],
                        )
                    hn = phc.tile([128, J], f32r, tag="h", name="hct")
                    evac(hn[:], ps[:], bias(4 + l), relu=True)
                    h_prev = hn

                # f1: 16-pair accumulation group -> lobes land densely at
                # psum row p%32 (see _prep_consts for the F1C layout).
                if i % 16 == 0:
                    psf1 = ppsf1.tile([32, J], f32, tag="psf1", name="psf1")
                v = i % 16
                for j in range(2):
                    nc.tensor.matmul(
                        psf1[:, j * N0:(j + 1) * N0],
                        F1CALL[:, 32 * v:32 * v + 32],
                        h_prev[:, j * N0:(j + 1) * N0],
                        start=(v == 0),
                        stop=(v == 15),
                        skip_group_check=True,
                    )
                if v == 15:
                    e = i // 16
                    scrap = pscrap.tile([32, J], f32r, tag="scrap", name="scrap")
                    if pick_act():
                        nc.scalar.activation(scrap[:], psf1[:], AF.Identity)
                    else:
                        nc.vector.tensor_copy(scrap[:], psf1[:])
                    nc.scalar.dma_start(out=LOB[32 * e:32 * e + 32, :],
                                        in_=scrap[:])

        # ================= Final: softplus + pdf =================
        # X = lobespre + f1_b; softplus(X) = max(X,0) + ln(1 + exp(-|X|))
        # (no Softplus activation table on this build; FEAT tiles are dead
        # here and reused as scratch)
        S0, S1 = FEAT[:, 0:J], FEAT[:, J:2 * J]
        nc.vector.tensor_scalar(LOB[:], LOB[:], bias(9), None, ALU.add)
        nc.vector.tensor_scalar(S0[:], LOB[:], -1.0, None, ALU.mult)
        nc.vector.tensor_tensor(S0[:], LOB[:], S0[:], ALU.min)
        nc.scalar.activation(S0[:], S0[:], AF.Exp)
        nc.scalar.activation(S0[:], S0[:], AF.Ln, bias=1.0)
        nc.vector.tensor_scalar_max(S1[:], LOB[:], 0.0)
        nc.vector.tensor_tensor(LOB[:], S0[:], S1[:], ALU.add)
        nc.vector.tensor_tensor(LOB[:], LOB[:], WPRE[:], ALU.mult)
        pspdf = pps.tile([128, J], f32, tag="ps", name="pspdf")
        for j in range(2):
            nc.tensor.matmul(
                pspdf[0:16, j * N0:(j + 1) * N0],
                SEL[:, :],
                LOB[:, j * N0:(j + 1) * N0],
            )
        nc.scalar.activation(PDFSB[:], pspdf[0:16, :], AF.Identity)
        pdf_r = pdf_out_d.ap().rearrange("(g j) -> g j", g=G)
        nc.sync.dma_start(out=pdf_r, in_=PDFSB[:])

    nc.compile()
    return nc


def _get_nc(J, debug=False):
    key = (J, debug)
    if key not in _BUILD_CACHE:
        _BUILD_CACHE[key] = _build(J, debug=debug)
    return _BUILD_CACHE[key]


def kernel(wi, cond, mlp0_ws, mlp0_bs, f0_ws, f0_bs, f1_w, f1_b):
    from concourse.bass_utils import run_bass_kernel_spmd

    wi = np.ascontiguousarray(np.asarray(wi, dtype=np.float32))
    cond = np.ascontiguousarray(np.asarray(cond, dtype=np.float32))
    B = wi.shape[0]
    Bc = B // NCORES
    J = Bc // G
    consts = _prep_consts(mlp0_ws, mlp0_bs, f0_ws, f0_bs, f1_w, f1_b)

    nc = _get_nc(J)
    in_maps = []
    for k in range(NCORES):
        m = {"wi": wi[k * Bc:(k + 1) * Bc], "cond": cond[k * Bc:(k + 1) * Bc]}
        m.update(consts)
        in_maps.append(m)
    global _LAST_RESULT
    res = run_bass_kernel_spmd(nc, in_maps, core_ids=list(range(NCORES)),
                               **_RUN_KWARGS)
    _LAST_RESULT = res
    cos_theta = np.concatenate([r["ct_out"] for r in res.results], axis=0)
    pdf = np.concatenate([r["pdf_out"] for r in res.results], axis=0)
    return cos_theta, pdf


# revision 32
# speedup vs baseline: 2.3161x; 2.1858x over previous
"""Trainium2 Bass kernel for nn_Base2D_80633716015315 (dense_mlp).

Math (per ray b, lobe d):
  feat = mlp0(cond)            # 10->32->32->32->32, relu between
  weight = relu(feat[:8]); v = feat[8:24] as [8,2]; ct = sigmoid(feat[24:32])
  v /= max(||v||, eps); wr = R(v) @ wi        # 2D rotation of wi by v
  x = [wr0, wr1, ct];  pe = [x, sin(2^k x), cos(2^k x)]  (27 feats)
  inter = relu(mlp(pe))        # 27->64->64->64->64->64, relu between + out
  lobes = softplus(inter . f1_w[d] + f1_b[d])
  pdf = sum_d lobes*weight
Returns (cos_theta [B,8], pdf [B]).

Sharding: pure data-parallel over B across 8 cores; weights replicated.

Per-core layout: Bc rows; partition index p = 8*g + d (g in 0..15 batch
groups, d in 0..7 lobes); free index j in 0..J-1; b = g*J + j.
Matmuls run feature-major (features on partitions) in float32r with
4-region tile_position packing; PSUM evacuation alternates ACT/DVE.
"""

import numpy as np

D = 8
G = 16
NCORES = 8

_BUILD_CACHE = {}
_RUN_KWARGS = {}
_LAST_RESULT = None


def _prep_consts(mlp0_ws, mlp0_bs, f0_ws, f0_bs, f1_w, f1_b):
    """Host-side constant tensors, all float32, laid out for the kernel."""
    f = lambda x: np.ascontiguousarray(np.asarray(x, dtype=np.float32))
    mlp0_ws = [f(w) for w in mlp0_ws]
    mlp0_bs = [f(b) for b in mlp0_bs]
    f0_ws = [f(w) for w in f0_ws]
    f0_bs = [f(b) for b in f0_bs]
    f1_w = f(f1_w)
    f1_b = f(f1_b)

    consts = {}
    # Permute the last mlp0 layer's output features so the blocks we need
    # later are contiguous: [v0(8), v1(8), wpre(8), ctpre(8)].
    perm = np.concatenate([
        np.arange(8, 24, 2),      # v0 = feat[8+2d]
        np.arange(9, 24, 2),      # v1 = feat[9+2d]
        np.arange(0, 8),          # weight pre-relu
        np.arange(24, 32),        # cos_theta pre-sigmoid
    ])
    w_last = mlp0_ws[3][perm, :]
    b_last = mlp0_bs[3][perm]

    # One packed lhsT bundle [9, 128, 128]:
    #  slots 0-3: mlp0 4-block-diagonal (layer0 K=40, else K=128)
    #  slots 4-8: f0 2-block-diagonal (layer0 K=54, else K=128)
    wall = np.zeros((9, 128, 128), np.float32)
    for l in range(4):
        w = w_last if l == 3 else mlp0_ws[l]
        K = w.shape[1]
        for q in range(4):
            wall[l, K * q:K * q + K, 32 * q:32 * q + 32] = w.T
    for l in range(5):
        w = f0_ws[l]
        K = w.shape[1]
        wall[4 + l, 0:K, 0:64] = w.T
        wall[4 + l, K:2 * K, 64:128] = w.T
    # layer-0 f0 block replicated at row 64 too (odd pairs read their PEX
    # block at base partition 64; matmul requires lhsT/rhs bases to match)
    w = f0_ws[0]
    wall[4, 64:64 + 27, 0:64] = w.T
    wall[4, 64 + 27:64 + 54, 64:128] = w.T
    consts["swall"] = wall.reshape(9 * 128, 128)

    # Bias bundle [128, 12]: cols 0-3 mlp0 (x4 repl), 4-8 f0 (x2 repl),
    # 9 f1 (f1_b[p%8]), 10 eps, 11 pi/2.
    ball = np.zeros((128, 12), np.float32)
    for l in range(4):
        b = b_last if l == 3 else mlp0_bs[l]
        ball[:, l] = np.tile(b, 4)
    for l in range(5):
        ball[:, 4 + l] = np.tile(f0_bs[l], 2)
    ball[:, 9] = np.tile(f1_b, G)
    ball[:, 10] = 1e-24
    ball[:, 11] = np.pi / 2
    consts["ball"] = ball

    # f1: 16 lhsT variants [128, 32]; variant v serves pairs i with i%16==v.
    # Column c=2v+h gets f1w[(2i+h)%8] in row-block h; other columns zero, so
    # a 32-matmul accumulation group lands lobes for 16 pairs densely in one
    # [32, J] psum tile (row p%32).
    f1c = np.zeros((16 * 128, 32), np.float32)
    for v in range(16):
        for h in range(2):
            d = (2 * v + h) % 8
            f1c[128 * v + 64 * h:128 * v + 64 * h + 64, 2 * v + h] = f1_w.T[:, d]
    consts["sf1c"] = f1c

    # pdf reduction selector: out[m] = sum_d PDFT[8m+d]
    sel = np.zeros((128, 16), np.float32)
    for p in range(128):
        sel[p, p // 8] = 1.0
    consts["sel"] = sel
    return consts


def _build(J, debug=False):
    """Build the single-core Bass program (run SPMD across all cores)."""
    from contextlib import ExitStack
    import concourse.bass as bass
    import concourse.mybir as mybir
    import concourse.tile as tile
    from concourse import bacc

    f32 = mybir.dt.float32
    f32r = mybir.dt.float32r
    AF = mybir.ActivationFunctionType
    ALU = mybir.AluOpType

    Bc = G * J
    N0 = J // 2
    NCH0 = Bc // N0          # 32 mlp0 chunks
    NPAIR = (G * D) // 2     # 64 stage-C pair iterations

    nc = bacc.Bacc("TRN2", target_bir_lowering=False, debug=debug)

    # ---- DRAM I/O (cond/wi are pre-laid-out on the host) ----
    condt_d = nc.dram_tensor("condt", [40, 4 * J], f32r, kind="ExternalInput")
    wixy_d = nc.dram_tensor("wixy", [128, 2 * J], f32, kind="ExternalInput")
    cdram = {}
    cshapes = {}
    for name, shape in [
        ("swall", (9 * 128, 128)),
        ("ball", (128, 12)),
        ("sf1c", (16 * 128, 32)),
        ("sel", (128, 16)),
    ]:
        wdt = f32r if name[0] == "s" else f32   # lhsT bundles f32r
        cdram[name] = nc.dram_tensor(name, list(shape), wdt, kind="ExternalInput")
        cshapes[name] = shape
    ct_out_d = nc.dram_tensor("ct_raw", [128, J], f32, kind="ExternalOutput")
    pdf_out_d = nc.dram_tensor("pdf_out", [Bc], f32, kind="ExternalOutput")
    featd = nc.dram_tensor("featd", [128, 4 * J], f32)   # HBM bounce

    # ACT/DVE duty split for psum evacuations (ACT also does sin/sigmoid etc)
    ACT_FRAC = 0.55
    evac_state = {"n": 0, "acts": 0}

    def pick_act():
        evac_state["n"] += 1
        want = int(evac_state["n"] * ACT_FRAC)
        if evac_state["acts"] < want:
            evac_state["acts"] += 1
            return True
        return False

    def evac(out_ap, in_ap, bias_ap, relu):
        if relu:
            if pick_act():
                nc.scalar.activation(out_ap, in_ap, AF.Relu, bias=bias_ap)
            else:
                nc.vector.tensor_scalar(out_ap, in_ap, bias_ap, 0.0,
                                        ALU.add, ALU.max)
        else:
            if pick_act():
                nc.scalar.activation(out_ap, in_ap, AF.Identity, bias=bias_ap)
            else:
                nc.vector.tensor_scalar(out_ap, in_ap, bias_ap, None, ALU.add)

    with tile.TileContext(nc) as tc, ExitStack() as ctx:
        pc = ctx.enter_context(tc.tile_pool(name="consts", bufs=1))
        pkeep = ctx.enter_context(tc.tile_pool(name="keep", bufs=1))
        pfeat = ctx.enter_context(tc.tile_pool(name="feat", bufs=1))
        pqall = ctx.enter_context(tc.tile_pool(name="qall", bufs=1))

        WALL = pc.tile([128, 9 * 128], f32r, tag="wall", name="wall")
        nc.sync.dma_start(
            out=WALL[:],
            in_=bass.AP(tensor=cdram["swall"].ap().tensor, offset=0,
                        ap=[[128, 128], [128 * 128, 9], [1, 128]]))
        BALL = pc.tile([128, 12], f32, tag="ball", name="ball")
        nc.sync.dma_start(out=BALL[:], in_=cdram["ball"].ap())
        F1CALL = pc.tile([128, 16 * 32], f32r, tag="f1call", name="f1call")
        nc.sync.dma_start(
            out=F1CALL[:],
            in_=bass.AP(tensor=cdram["sf1c"].ap().tensor, offset=0,
                        ap=[[32, 128], [128 * 32, 16], [1, 32]]))
        SEL = pc.tile([128, 16], f32r, tag="sel", name="sel")
        nc.sync.dma_start(out=SEL[:], in_=cdram["sel"].ap())

        def wmat(slot, K):
            return WALL[0:K, 128 * slot:128 * slot + 128]

        def bias(col):
            return BALL[:, col:col + 1]

        WPRE = pkeep.tile([128, J], f32, tag="wpre")    # -> WEIGHT (in place)
        LOB = pkeep.tile([128, J], f32r, tag="lob")      # lobespre -> lobes -> pdft
        PDFSB = pkeep.tile([16, J], f32, tag="pdfsb")

        FEAT = pfeat.tile([128, 4 * J], f32, tag="feat", name="feat")
        QALL = pqall.tile([128, 27 * J], f32r, tag="qall")

        pps = ctx.enter_context(tc.tile_pool(name="ps", bufs=3, space="PSUM"))
        ppsf1 = ctx.enter_context(tc.tile_pool(name="psf1", bufs=1, space="PSUM"))

        # ================= Stage A: mlp0 (4-block-diagonal) =================
        with tc.tile_pool(name="mlp0io", bufs=1) as pm, \
             tc.tile_pool(name="h0", bufs=2) as ph0:
            CONDT = pm.tile([40, 8 * N0], f32r, tag="condt", name="condt")
            nc.sync.dma_start(out=CONDT[:], in_=condt_d.ap())

            KS0 = [40, 128, 128, 128]
            for jp in range(4):                 # 4 pairs of chunk-waves
                h_prev = None
                for l in range(4):
                    ps = pps.tile([128, J], f32, tag="ps", name="ps")
                    K = KS0[l]
                    for c in range(2):
                        if l == 0:
                            rhs = CONDT[:, (2 * jp + c) * N0:(2 * jp + c + 1) * N0]
                        else:
                            rhs = h_prev[:, c * N0:(c + 1) * N0]
                        nc.tensor.matmul(
                            ps[:, c * N0:(c + 1) * N0],
                            wmat(l, K),
                            rhs,
                        )
                    if l < 3:
                        h = ph0.tile([128, J], f32r, tag="h0", name="h0t")
                        evac(h[:], ps[:], bias(l), relu=True)
                        h_prev = h
                    else:
                        evac(FEAT[:, jp * J:(jp + 1) * J], ps[:], bias(l),
                             relu=False)

        # ============ V-shuffle + Stage B (elementwise) ============
        with tc.tile_pool(name="bvars", bufs=1) as pb:
            V0 = pb.tile([128, J], f32, tag="v0")
            V1 = pb.tile([128, J], f32, tag="v1")
            CTP = pb.tile([128, J], f32, tag="ctp")
            WIXY = pb.tile([128, 2 * J], f32, tag="wixy", name="wixy")
            T0 = pb.tile([128, J], f32, tag="t0")
            T1 = pb.tile([128, J], f32, tag="t1")

            # V-shuffle via HBM bounce: FEAT -> featd (contiguous), then
            # one read per (quantity, b-block): dst partitions 32b..32b+32
            # (p = 32b+8jp+d, sequential = (jp,d) order), 3-dim DRAM src.
            nc.sync.dma_start(out=featd.ap(), in_=FEAT[:])
            for qty, dst in ((0, V0), (1, V1), (2, WPRE), (3, CTP)):
                for b_blk in range(4):
                    src = bass.AP(tensor=featd.ap().tensor,
                                  offset=(32 * b_blk + 8 * qty) * (4 * J),
                                  ap=[[J, 4], [4 * J, 8], [1, J]])
                    eng = nc.sync if qty % 2 == 0 else nc.scalar
                    eng.dma_start(out=dst[32 * b_blk:32 * b_blk + 32, :],
                                  in_=src)

            # wi: host-prepped broadcast layout, one DMA
            nc.sync.dma_start(out=WIXY[:], in_=wixy_d.ap())
            WIX = WIXY[:, 0:J]
            WIY = WIXY[:, J:2 * J]

            wr0 = QALL[:, 0:J]
            wr1 = QALL[:, J:2 * J]
            ct = QALL[:, 2 * J:3 * J]

            # rinv = exp(-0.5*ln(v0^2+v1^2+eps))
            nc.vector.tensor_tensor(T0[:], V0[:], V0[:], ALU.mult)
            nc.vector.tensor_tensor(T1[:], V1[:], V1[:], ALU.mult)
            nc.vector.tensor_tensor(T0[:], T0[:], T1[:], ALU.add)
            nc.scalar.activation(T1[:], T0[:], AF.Ln, bias=bias(10))
            nc.scalar.activation(T0[:], T1[:], AF.Exp, scale=-0.5)
            # normalize v
            nc.vector.tensor_tensor(V0[:], V0[:], T0[:], ALU.mult)
            nc.vector.tensor_tensor(V1[:], V1[:], T0[:], ALU.mult)
            # wr = R(v) wi
            nc.vector.tensor_tensor(T0[:], V0[:], WIX, ALU.mult)
            nc.vector.tensor_tensor(T1[:], V1[:], WIY, ALU.mult)
            nc.vector.tensor_tensor(wr0, T0[:], T1[:], ALU.subtract)
            nc.vector.tensor_tensor(T0[:], V1[:], WIX, ALU.mult)
            nc.vector.tensor_tensor(T1[:], V0[:], WIY, ALU.mult)
            nc.vector.tensor_tensor(wr1, T0[:], T1[:], ALU.add)
            # ct = sigmoid(ctpre)
            nc.scalar.activation(ct, CTP[:], AF.Sigmoid)
            # weight = relu(wpre), in place
            nc.vector.tensor_scalar_max(WPRE[:], WPRE[:], 0.0)
            # positional encoding. ACT Sin only accepts [-pi, pi]; k=0
            # inputs are in range (|wr|<=1, ct in [0,1], +pi/2 <= 2.58), so
            # compute sin/cos at k=0 on ACT and k=1..3 by double-angle
            # recurrences (sin2x=2 s c, cos2x=1-2 s^2) on DVE/GPSIMD.
            T2 = pb.tile([128, J], f32, tag="t2", name="t2")
            T3 = pb.tile([128, J], f32, tag="t3", name="t3")

            def sinf(k, ci):
                f = 3 + 3 * k + ci
                return QALL[:, f * J:(f + 1) * J]

            def cosf(k, ci):
                f = 15 + 3 * k + ci
                return QALL[:, f * J:(f + 1) * J]

            for ci, comp in enumerate((wr0, wr1, ct)):
                nc.scalar.activation(sinf(0, ci), comp, AF.Sin)
                nc.scalar.activation(cosf(0, ci), comp, AF.Sin, bias=bias(11))
            for ci in range(3):
                eng = nc.gpsimd if ci == 1 else nc.vector
                ta, tb = (T2, T3) if ci == 1 else (T0, T1)
                for k in range(3):
                    s_k, c_k = sinf(k, ci), cosf(k, ci)
                    eng.tensor_tensor(ta[:], s_k, c_k, ALU.mult)
                    nc.vector.tensor_scalar(sinf(k + 1, ci), ta[:], 2.0,
                                            None, ALU.mult)
                    eng.tensor_tensor(tb[:], s_k, s_k, ALU.mult)
                    nc.vector.tensor_scalar(cosf(k + 1, ci), tb[:], -2.0,
                                            1.0, ALU.mult, ALU.add)

            # cos_theta: contiguous raw dump; host reshapes to [Bc, 8]
            nc.sync.dma_start(out=ct_out_d.ap(),
                              in_=QALL[:, 2 * J:3 * J].bitcast(f32))

        # ================= Stage C: f0 MLP + f1 =================
        # Per pair i: streams A=p0 (feature rows 0-63) and B=p1 (64-127),
        # block-diagonal lhsT, full-array matmuls, no tile_position. PEX
        # tiles hold 2 pairs (4 p's, [108, J]) loaded by one DMA each.
        KSF = [54, 128, 128, 128, 128]
        with tc.tile_pool(name="pex", bufs=3) as ppex, \
             tc.tile_pool(name="hc", bufs=6) as phc, \
             tc.tile_pool(name="scrap", bufs=2) as pscrap:
            psf1 = None
            pex2 = None
            qpitch = QALL.ap[0][0]
            for i in range(NPAIR):
                if i % 2 == 0:
                    pex2 = ppex.tile([128, J], f32r, tag="pex", name="pex")
                    for half in range(2):
                        src = bass.AP(
                            tensor=QALL.tensor,
                            offset=QALL.offset + (2 * i + 2 * half) * qpitch,
                            ap=[[qpitch, 2], [J, 27], [1, J]])
                        eng = nc.sync if (i // 2 + half) % 2 == 0 else nc.scalar
                        eng.dma_start(
                            out=pex2[64 * half:64 * half + 54, :], in_=src)
                pex = pex2[64 * (i % 2):64 * (i % 2) + 54, :]

                h_prev = pex
                base0 = 64 * (i % 2)
                for l in range(5):
                    K = KSF[l]
                    base = base0 if l == 0 else 0
                    ps = pps.tile([128, J], f32, tag="ps", name="ps")
                    for j in range(2):
                        nc.tensor.matmul(
                            ps[:, j * N0:(j + 1) * N0],
                            WALL[base:base + K,
                                 128 * (4 + l):128 * (4 + l) + 128],
                            h_prev[0:K, j * N0:(j + 1) * N0]
                            if l > 0 else
                            pex2[base:base + K, j * N0:(j + 1) * N0],
                        )
                    hn = phc.tile([128, J], f32r, tag="h", name="hct")
                    evac(hn[:], ps[:], bias(4 + l), relu=True)
                    h_prev = hn

                # f1: 16-pair accumulation group -> lobes land densely at
                # psum row p%32 (see _prep_consts for the F1C layout).
                if i % 16 == 0:
                    psf1 = ppsf1.tile([32, J], f32, tag="psf1", name="psf1")
                v = i % 16
                for j in range(2):
                    nc.tensor.matmul(
                        psf1[:, j * N0:(j + 1) * N0],
                        F1CALL[:, 32 * v:32 * v + 32],
                        h_prev[:, j * N0:(j + 1) * N0],
                        start=(v == 0),
                        stop=(v == 15),
                        skip_group_check=True,
                    )
                if v == 15:
                    e = i // 16
                    scrap = pscrap.tile([32, J], f32r, tag="scrap", name="scrap")
                    if pick_act():
                        nc.scalar.activation(scrap[:], psf1[:], AF.Identity)
                    else:
                        nc.vector.tensor_copy(scrap[:], psf1[:])
                    nc.scalar.dma_start(out=LOB[32 * e:32 * e + 32, :],
                                        in_=scrap[:])

        # ================= Final: softplus + pdf =================
        # X = lobespre + f1_b; softplus(X) = max(X,0) + ln(1 + exp(-|X|))
        # (no Softplus activation table on this build; FEAT tiles are dead
        # here and reused as scratch)
        S0, S1 = FEAT[:, 0:J], FEAT[:, J:2 * J]
        nc.vector.tensor_scalar(LOB[:], LOB[:], bias(9), None, ALU.add)
        nc.vector.tensor_scalar(S0[:], LOB[:], -1.0, None, ALU.mult)
        nc.vector.tensor_tensor(S0[:], LOB[:], S0[:], ALU.min)
        nc.scalar.activation(S0[:], S0[:], AF.Exp)
        nc.scalar.activation(S0[:], S0[:], AF.Ln, bias=1.0)
        nc.vector.tensor_scalar_max(S1[:], LOB[:], 0.0)
        nc.vector.tensor_tensor(LOB[:], S0[:], S1[:], ALU.add)
        nc.vector.tensor_tensor(LOB[:], LOB[:], WPRE[:], ALU.mult)
        pspdf = pps.tile([128, J], f32, tag="ps", name="pspdf")
        for j in range(2):
            nc.tensor.matmul(
                pspdf[0:16, j * N0:(j + 1) * N0],
                SEL[:, :],
                LOB[:, j * N0:(j + 1) * N0],
            )
        nc.scalar.activation(PDFSB[:], pspdf[0:16, :], AF.Identity)
        pdf_r = pdf_out_d.ap().rearrange("(g j) -> g j", g=G)
        nc.sync.dma_start(out=pdf_r, in_=PDFSB[:])

    nc.compile()
    return nc


def _get_nc(J, debug=False):
    key = (J, debug)
    if key not in _BUILD_CACHE:
        _BUILD_CACHE[key] = _build(J, debug=debug)
    return _BUILD_CACHE[key]


def _prep_core_inputs(wi_c, cond_c, J):
    """Host-side layout staging for one core's wi/cond slices."""
    # condt[10q+f, b'] = cond[q*4J + b', f]
    condt = np.ascontiguousarray(
        cond_c.reshape(4, 4 * J, 10).transpose(0, 2, 1).reshape(40, 4 * J))
    # wixy[8g+d, comp*J+j] = wi[g*J+j, comp]  (d broadcast)
    w = wi_c.reshape(G, J, 2).transpose(0, 2, 1).reshape(G, 1, 2 * J)
    wixy = np.ascontiguousarray(np.broadcast_to(w, (G, 8, 2 * J))
                                .reshape(128, 2 * J))
    return {"condt": condt, "wixy": wixy}


def _unpack_ct(raw, J):
    """ct_raw[8g+d, j] -> cos_theta[g*J+j, d]."""
    return np.ascontiguousarray(
        raw.reshape(G, 8, J).transpose(0, 2, 1).reshape(G * J, 8))


def kernel(wi, cond, mlp0_ws, mlp0_bs, f0_ws, f0_bs, f1_w, f1_b):
    from concourse.bass_utils import run_bass_kernel_spmd

    wi = np.ascontiguousarray(np.asarray(wi, dtype=np.float32))
    cond = np.ascontiguousarray(np.asarray(cond, dtype=np.float32))
    B = wi.shape[0]
    Bc = B // NCORES
    J = Bc // G
    consts = _prep_consts(mlp0_ws, mlp0_bs, f0_ws, f0_bs, f1_w, f1_b)

    nc = _get_nc(J)
    in_maps = []
    for k in range(NCORES):
        m = _prep_core_inputs(wi[k * Bc:(k + 1) * Bc],
                              cond[k * Bc:(k + 1) * Bc], J)
        m.update(consts)
        in_maps.append(m)
    global _LAST_RESULT
    res = run_bass_kernel_spmd(nc, in_maps, core_ids=list(range(NCORES)),
                               **_RUN_KWARGS)
    _LAST_RESULT = res
    cos_theta = np.concatenate([_unpack_ct(r["ct_raw"], J)
                                for r in res.results], axis=0)
    pdf = np.concatenate([r["pdf_out"] for r in res.results], axis=0)
    return cos_theta, pdf


# revision 33
# speedup vs baseline: 2.5266x; 1.0909x over previous
"""Trainium2 Bass kernel for nn_Base2D_80633716015315 (dense_mlp).

Math (per ray b, lobe d):
  feat = mlp0(cond)            # 10->32->32->32->32, relu between
  weight = relu(feat[:8]); v = feat[8:24] as [8,2]; ct = sigmoid(feat[24:32])
  v /= max(||v||, eps); wr = R(v) @ wi        # 2D rotation of wi by v
  x = [wr0, wr1, ct];  pe = [x, sin(2^k x), cos(2^k x)]  (27 feats)
  inter = relu(mlp(pe))        # 27->64->64->64->64->64, relu between + out
  lobes = softplus(inter . f1_w[d] + f1_b[d])
  pdf = sum_d lobes*weight
Returns (cos_theta [B,8], pdf [B]).

Sharding: pure data-parallel over B across 8 cores; weights replicated.

Per-core layout: Bc rows; partition index p = 8*g + d (g in 0..15 batch
groups, d in 0..7 lobes); free index j in 0..J-1; b = g*J + j.
Matmuls run feature-major (features on partitions) in float32r with
4-region tile_position packing; PSUM evacuation alternates ACT/DVE.
"""

import numpy as np

D = 8
G = 16
NCORES = 8

_BUILD_CACHE = {}
_RUN_KWARGS = {}
_LAST_RESULT = None


def _prep_consts(mlp0_ws, mlp0_bs, f0_ws, f0_bs, f1_w, f1_b):
    """Host-side constant tensors, all float32, laid out for the kernel."""
    f = lambda x: np.ascontiguousarray(np.asarray(x, dtype=np.float32))
    mlp0_ws = [f(w) for w in mlp0_ws]
    mlp0_bs = [f(b) for b in mlp0_bs]
    f0_ws = [f(w) for w in f0_ws]
    f0_bs = [f(b) for b in f0_bs]
    f1_w = f(f1_w)
    f1_b = f(f1_b)

    consts = {}
    # Permute the last mlp0 layer's output features so the blocks we need
    # later are contiguous: [v0(8), v1(8), wpre(8), ctpre(8)].
    perm = np.concatenate([
        np.arange(8, 24, 2),      # v0 = feat[8+2d]
        np.arange(9, 24, 2),      # v1 = feat[9+2d]
        np.arange(0, 8),          # weight pre-relu
        np.arange(24, 32),        # cos_theta pre-sigmoid
    ])
    w_last = mlp0_ws[3][perm, :]
    b_last = mlp0_bs[3][perm]

    # One packed lhsT bundle [9, 128, 128]:
    #  slots 0-3: mlp0 4-block-diagonal (layer0 K=40, else K=128)
    #  slots 4-8: f0 2-block-diagonal (layer0 K=54, else K=128)
    wall = np.zeros((9, 128, 128), np.float32)
    for l in range(4):
        w = w_last if l == 3 else mlp0_ws[l]
        K = w.shape[1]
        for q in range(4):
            wall[l, K * q:K * q + K, 32 * q:32 * q + 32] = w.T
    for l in range(5):
        w = f0_ws[l]
        K = w.shape[1]
        wall[4 + l, 0:K, 0:64] = w.T
        wall[4 + l, K:2 * K, 64:128] = w.T
    # layer-0 f0 block replicated at row 64 too (odd pairs read their PEX
    # block at base partition 64; matmul requires lhsT/rhs bases to match)
    w = f0_ws[0]
    wall[4, 64:64 + 27, 0:64] = w.T
    wall[4, 64 + 27:64 + 54, 64:128] = w.T
    import ml_dtypes
    consts["swall"] = wall.reshape(9 * 128, 128).astype(ml_dtypes.bfloat16)

    # Bias bundle [128, 12]: cols 0-3 mlp0 (x4 repl), 4-8 f0 (x2 repl),
    # 9 f1 (f1_b[p%8]), 10 eps, 11 pi/2.
    ball = np.zeros((128, 12), np.float32)
    for l in range(4):
        b = b_last if l == 3 else mlp0_bs[l]
        ball[:, l] = np.tile(b, 4)
    for l in range(5):
        ball[:, 4 + l] = np.tile(f0_bs[l], 2)
    ball[:, 9] = np.tile(f1_b, G)
    ball[:, 10] = 1e-24
    ball[:, 11] = np.pi / 2
    consts["ball"] = ball

    # f1: 16 lhsT variants [128, 32]; variant v serves pairs i with i%16==v.
    # Column c=2v+h gets f1w[(2i+h)%8] in row-block h; other columns zero, so
    # a 32-matmul accumulation group lands lobes for 16 pairs densely in one
    # [32, J] psum tile (row p%32).
    f1c = np.zeros((16 * 128, 32), np.float32)
    for v in range(16):
        for h in range(2):
            d = (2 * v + h) % 8
            f1c[128 * v + 64 * h:128 * v + 64 * h + 64, 2 * v + h] = f1_w.T[:, d]
    consts["sf1c"] = f1c.astype(ml_dtypes.bfloat16)

    # pdf reduction selector: out[m] = sum_d PDFT[8m+d]
    sel = np.zeros((128, 16), np.float32)
    for p in range(128):
        sel[p, p // 8] = 1.0
    consts["sel"] = sel
    return consts


def _build(J, debug=False):
    """Build the single-core Bass program (run SPMD across all cores)."""
    from contextlib import ExitStack
    import concourse.bass as bass
    import concourse.mybir as mybir
    import concourse.tile as tile
    from concourse import bacc

    f32 = mybir.dt.float32
    f32r = mybir.dt.float32r
    bf16 = mybir.dt.bfloat16
    AF = mybir.ActivationFunctionType
    ALU = mybir.AluOpType

    Bc = G * J
    N0 = J // 2
    NCH0 = Bc // N0          # 32 mlp0 chunks
    NPAIR = (G * D) // 2     # 64 stage-C pair iterations

    nc = bacc.Bacc("TRN2", target_bir_lowering=False, debug=debug)

    # ---- DRAM I/O (cond/wi are pre-laid-out on the host) ----
    condt_d = nc.dram_tensor("condt", [40, 4 * J], bf16, kind="ExternalInput")
    wixy_d = nc.dram_tensor("wixy", [128, 2 * J], f32, kind="ExternalInput")
    cdram = {}
    cshapes = {}
    for name, shape in [
        ("swall", (9 * 128, 128)),
        ("ball", (128, 12)),
        ("sf1c", (16 * 128, 32)),
        ("sel", (128, 16)),
    ]:
        wdt = {"swall": bf16, "sf1c": bf16, "sel": f32r, "ball": f32}[name]
        cdram[name] = nc.dram_tensor(name, list(shape), wdt, kind="ExternalInput")
        cshapes[name] = shape
    ct_out_d = nc.dram_tensor("ct_raw", [128, J], f32, kind="ExternalOutput")
    pdf_out_d = nc.dram_tensor("pdf_out", [Bc], f32, kind="ExternalOutput")
    featd = nc.dram_tensor("featd", [128, 4 * J], f32)   # HBM bounce

    # ACT/DVE duty split for psum evacuations (ACT also does sin/sigmoid etc)
    ACT_FRAC = 0.55
    evac_state = {"n": 0, "acts": 0}

    def pick_act():
        evac_state["n"] += 1
        want = int(evac_state["n"] * ACT_FRAC)
        if evac_state["acts"] < want:
            evac_state["acts"] += 1
            return True
        return False

    def evac(out_ap, in_ap, bias_ap, relu):
        if relu:
            if pick_act():
                nc.scalar.activation(out_ap, in_ap, AF.Relu, bias=bias_ap)
            else:
                nc.vector.tensor_scalar(out_ap, in_ap, bias_ap, 0.0,
                                        ALU.add, ALU.max)
        else:
            if pick_act():
                nc.scalar.activation(out_ap, in_ap, AF.Identity, bias=bias_ap)
            else:
                nc.vector.tensor_scalar(out_ap, in_ap, bias_ap, None, ALU.add)

    with tile.TileContext(nc) as tc, ExitStack() as ctx:
        pc = ctx.enter_context(tc.tile_pool(name="consts", bufs=1))
        pkeep = ctx.enter_context(tc.tile_pool(name="keep", bufs=1))
        pfeat = ctx.enter_context(tc.tile_pool(name="feat", bufs=1))
        pqall = ctx.enter_context(tc.tile_pool(name="qall", bufs=1))

        WALL = pc.tile([128, 9 * 128], bf16, tag="wall", name="wall")
        nc.sync.dma_start(
            out=WALL[:],
            in_=bass.AP(tensor=cdram["swall"].ap().tensor, offset=0,
                        ap=[[128, 128], [128 * 128, 9], [1, 128]]))
        BALL = pc.tile([128, 12], f32, tag="ball", name="ball")
        nc.sync.dma_start(out=BALL[:], in_=cdram["ball"].ap())
        F1CALL = pc.tile([128, 16 * 32], bf16, tag="f1call", name="f1call")
        nc.sync.dma_start(
            out=F1CALL[:],
            in_=bass.AP(tensor=cdram["sf1c"].ap().tensor, offset=0,
                        ap=[[32, 128], [128 * 32, 16], [1, 32]]))
        SEL = pc.tile([128, 16], f32r, tag="sel", name="sel")
        nc.sync.dma_start(out=SEL[:], in_=cdram["sel"].ap())

        def wmat(slot, K):
            return WALL[0:K, 128 * slot:128 * slot + 128]

        def bias(col):
            return BALL[:, col:col + 1]

        WPRE = pkeep.tile([128, J], f32, tag="wpre")    # -> WEIGHT (in place)
        CTOUT = pkeep.tile([128, J], f32, tag="ctout")  # fp32 cos_theta out
        LOB = pkeep.tile([128, J], f32r, tag="lob")      # lobespre -> lobes -> pdft
        PDFSB = pkeep.tile([16, J], f32, tag="pdfsb")

        FEAT = pfeat.tile([128, 4 * J], f32, tag="feat", name="feat")
        QALL = pqall.tile([128, 27 * J], bf16, tag="qall")

        pps = ctx.enter_context(tc.tile_pool(name="ps", bufs=3, space="PSUM"))
        ppsf1 = ctx.enter_context(tc.tile_pool(name="psf1", bufs=1, space="PSUM"))

        # ================= Stage A: mlp0 (4-block-diagonal) =================
        with tc.tile_pool(name="mlp0io", bufs=1) as pm, \
             tc.tile_pool(name="h0", bufs=2) as ph0:
            CONDT = pm.tile([40, 8 * N0], bf16, tag="condt", name="condt")
            nc.sync.dma_start(out=CONDT[:], in_=condt_d.ap())

            KS0 = [40, 128, 128, 128]
            for jp in range(4):                 # 4 pairs of chunk-waves
                h_prev = None
                for l in range(4):
                    ps = pps.tile([128, J], f32, tag="ps", name="ps")
                    K = KS0[l]
                    for c in range(2):
                        if l == 0:
                            rhs = CONDT[:, (2 * jp + c) * N0:(2 * jp + c + 1) * N0]
                        else:
                            rhs = h_prev[:, c * N0:(c + 1) * N0]
                        nc.tensor.matmul(
                            ps[:, c * N0:(c + 1) * N0],
                            wmat(l, K),
                            rhs,
                        )
                    if l < 3:
                        h = ph0.tile([128, J], bf16, tag="h0", name="h0t")
                        evac(h[:], ps[:], bias(l), relu=True)
                        h_prev = h
                    else:
                        evac(FEAT[:, jp * J:(jp + 1) * J], ps[:], bias(l),
                             relu=False)

        # ============ V-shuffle + Stage B (elementwise) ============
        with tc.tile_pool(name="bvars", bufs=1) as pb:
            V0 = pb.tile([128, J], f32, tag="v0")
            V1 = pb.tile([128, J], f32, tag="v1")
            CTP = pb.tile([128, J], f32, tag="ctp")
            WIXY = pb.tile([128, 2 * J], f32, tag="wixy", name="wixy")
            T0 = pb.tile([128, J], f32, tag="t0")
            T1 = pb.tile([128, J], f32, tag="t1")

            # V-shuffle via HBM bounce: FEAT -> featd (contiguous), then
            # one read per (quantity, b-block): dst partitions 32b..32b+32
            # (p = 32b+8jp+d, sequential = (jp,d) order), 3-dim DRAM src.
            nc.sync.dma_start(out=featd.ap(), in_=FEAT[:])
            for qty, dst in ((0, V0), (1, V1), (2, WPRE), (3, CTP)):
                for b_blk in range(4):
                    src = bass.AP(tensor=featd.ap().tensor,
                                  offset=(32 * b_blk + 8 * qty) * (4 * J),
                                  ap=[[J, 4], [4 * J, 8], [1, J]])
                    eng = nc.sync if qty % 2 == 0 else nc.scalar
                    eng.dma_start(out=dst[32 * b_blk:32 * b_blk + 32, :],
                                  in_=src)

            # wi: host-prepped broadcast layout, one DMA
            nc.sync.dma_start(out=WIXY[:], in_=wixy_d.ap())
            WIX = WIXY[:, 0:J]
            WIY = WIXY[:, J:2 * J]

            wr0 = QALL[:, 0:J]
            wr1 = QALL[:, J:2 * J]
            ct = QALL[:, 2 * J:3 * J]

            # rinv = exp(-0.5*ln(v0^2+v1^2+eps))
            nc.vector.tensor_tensor(T0[:], V0[:], V0[:], ALU.mult)
            nc.vector.tensor_tensor(T1[:], V1[:], V1[:], ALU.mult)
            nc.vector.tensor_tensor(T0[:], T0[:], T1[:], ALU.add)
            nc.scalar.activation(T1[:], T0[:], AF.Ln, bias=bias(10))
            nc.scalar.activation(T0[:], T1[:], AF.Exp, scale=-0.5)
            # normalize v
            nc.vector.tensor_tensor(V0[:], V0[:], T0[:], ALU.mult)
            nc.vector.tensor_tensor(V1[:], V1[:], T0[:], ALU.mult)
            # wr = R(v) wi
            nc.vector.tensor_tensor(T0[:], V0[:], WIX, ALU.mult)
            nc.vector.tensor_tensor(T1[:], V1[:], WIY, ALU.mult)
            nc.vector.tensor_tensor(wr0, T0[:], T1[:], ALU.subtract)
            nc.vector.tensor_tensor(T0[:], V1[:], WIX, ALU.mult)
            nc.vector.tensor_tensor(T1[:], V0[:], WIY, ALU.mult)
            nc.vector.tensor_tensor(wr1, T0[:], T1[:], ALU.add)
            # ct = sigmoid(ctpre): bf16 copy feeds the PE features, fp32
            # copy is the returned output
            nc.scalar.activation(ct, CTP[:], AF.Sigmoid)
            nc.scalar.activation(CTOUT[:], CTP[:], AF.Sigmoid)
            # weight = relu(wpre), in place
            nc.vector.tensor_scalar_max(WPRE[:], WPRE[:], 0.0)
            # positional encoding. ACT Sin only accepts [-pi, pi]; k=0
            # inputs are in range (|wr|<=1, ct in [0,1], +pi/2 <= 2.58), so
            # compute sin/cos at k=0 on ACT and k=1..3 by double-angle
            # recurrences (sin2x=2 s c, cos2x=1-2 s^2) on DVE/GPSIMD.
            T2 = pb.tile([128, J], f32, tag="t2", name="t2")
            T3 = pb.tile([128, J], f32, tag="t3", name="t3")

            def sinf(k, ci):
                f = 3 + 3 * k + ci
                return QALL[:, f * J:(f + 1) * J]

            def cosf(k, ci):
                f = 15 + 3 * k + ci
                return QALL[:, f * J:(f + 1) * J]

            for ci, comp in enumerate((wr0, wr1, ct)):
                nc.scalar.activation(sinf(0, ci), comp, AF.Sin)
                nc.scalar.activation(cosf(0, ci), comp, AF.Sin, bias=bias(11))
            for ci in range(3):
                eng = nc.gpsimd if ci == 1 else nc.vector
                ta, tb = (T2, T3) if ci == 1 else (T0, T1)
                for k in range(3):
                    s_k, c_k = sinf(k, ci), cosf(k, ci)
                    eng.tensor_tensor(ta[:], s_k, c_k, ALU.mult)
                    nc.vector.tensor_scalar(sinf(k + 1, ci), ta[:], 2.0,
                                            None, ALU.mult)
                    eng.tensor_tensor(tb[:], s_k, s_k, ALU.mult)
                    nc.vector.tensor_scalar(cosf(k + 1, ci), tb[:], -2.0,
                                            1.0, ALU.mult, ALU.add)

            # cos_theta: contiguous raw dump; host reshapes to [Bc, 8]
            nc.sync.dma_start(out=ct_out_d.ap(), in_=CTOUT[:])

        # ================= Stage C: f0 MLP + f1 =================
        # Per pair i: streams A=p0 (feature rows 0-63) and B=p1 (64-127),
        # block-diagonal lhsT, full-array matmuls, no tile_position. PEX
        # tiles hold 2 pairs (4 p's, [108, J]) loaded by one DMA each.
        KSF = [54, 128, 128, 128, 128]
        with tc.tile_pool(name="pex", bufs=4) as ppex, \
             tc.tile_pool(name="hc", bufs=8) as phc, \
             tc.tile_pool(name="scrap", bufs=2) as pscrap:
            psf1 = None
            pex2 = None
            qpitch = QALL.ap[0][0]
            for i in range(NPAIR):
                if i % 2 == 0:
                    pex2 = ppex.tile([128, J], bf16, tag="pex", name="pex")
                    for half in range(2):
                        src = bass.AP(
                            tensor=QALL.tensor,
                            offset=QALL.offset + (2 * i + 2 * half) * qpitch,
                            ap=[[qpitch, 2], [J, 27], [1, J]])
                        eng = nc.sync if (i // 2 + half) % 2 == 0 else nc.scalar
                        eng.dma_start(
                            out=pex2[64 * half:64 * half + 54, :], in_=src)
                pex = pex2[64 * (i % 2):64 * (i % 2) + 54, :]

                h_prev = pex
                base0 = 64 * (i % 2)
                for l in range(5):
                    K = KSF[l]
                    base = base0 if l == 0 else 0
                    ps = pps.tile([128, J], f32, tag="ps", name="ps")
                    for j in range(2):
                        nc.tensor.matmul(
                            ps[:, j * N0:(j + 1) * N0],
                            WALL[base:base + K,
                                 128 * (4 + l):128 * (4 + l) + 128],
                            h_prev[0:K, j * N0:(j + 1) * N0]
                            if l > 0 else
                            pex2[base:base + K, j * N0:(j + 1) * N0],
                        )
                    hn = phc.tile([128, J], bf16, tag="h", name="hct")
                    evac(hn[:], ps[:], bias(4 + l), relu=True)
                    h_prev = hn

                # f1: 16-pair accumulation group -> lobes land densely at
                # psum row p%32 (see _prep_consts for the F1C layout).
                if i % 16 == 0:
                    psf1 = ppsf1.tile([32, J], f32, tag="psf1", name="psf1")
                v = i % 16
                for j in range(2):
                    nc.tensor.matmul(
                        psf1[:, j * N0:(j + 1) * N0],
                        F1CALL[:, 32 * v:32 * v + 32],
                        h_prev[:, j * N0:(j + 1) * N0],
                        start=(v == 0),
                        stop=(v == 15),
                        skip_group_check=True,
                    )
                if v == 15:
                    e = i // 16
                    scrap = pscrap.tile([32, J], f32r, tag="scrap", name="scrap")
                    if pick_act():
                        nc.scalar.activation(scrap[:], psf1[:], AF.Identity)
                    else:
                        nc.vector.tensor_copy(scrap[:], psf1[:])
                    nc.scalar.dma_start(out=LOB[32 * e:32 * e + 32, :],
                                        in_=scrap[:])

        # ================= Final: softplus + pdf =================
        # X = lobespre + f1_b; softplus(X) = max(X,0) + ln(1 + exp(-|X|))
        # (no Softplus activation table on this build; FEAT tiles are dead
        # here and reused as scratch)
        S0, S1 = FEAT[:, 0:J], FEAT[:, J:2 * J]
        nc.vector.tensor_scalar(LOB[:], LOB[:], bias(9), None, ALU.add)
        nc.vector.tensor_scalar(S0[:], LOB[:], -1.0, None, ALU.mult)
        nc.vector.tensor_tensor(S0[:], LOB[:], S0[:], ALU.min)
        nc.scalar.activation(S0[:], S0[:], AF.Exp)
        nc.scalar.activation(S0[:], S0[:], AF.Ln, bias=1.0)
        nc.vector.tensor_scalar_max(S1[:], LOB[:], 0.0)
        nc.vector.tensor_tensor(LOB[:], S0[:], S1[:], ALU.add)
        nc.vector.tensor_tensor(LOB[:], LOB[:], WPRE[:], ALU.mult)
        pspdf = pps.tile([128, J], f32, tag="ps", name="pspdf")
        for j in range(2):
            nc.tensor.matmul(
                pspdf[0:16, j * N0:(j + 1) * N0],
                SEL[:, :],
                LOB[:, j * N0:(j + 1) * N0],
            )
        nc.scalar.activation(PDFSB[:], pspdf[0:16, :], AF.Identity)
        pdf_r = pdf_out_d.ap().rearrange("(g j) -> g j", g=G)
        nc.sync.dma_start(out=pdf_r, in_=PDFSB[:])

    nc.compile()
    return nc


def _get_nc(J, debug=False):
    key = (J, debug)
    if key not in _BUILD_CACHE:
        _BUILD_CACHE[key] = _build(J, debug=debug)
    return _BUILD_CACHE[key]


def _prep_core_inputs(wi_c, cond_c, J):
    """Host-side layout staging for one core's wi/cond slices."""
    # condt[10q+f, b'] = cond[q*4J + b', f]
    import ml_dtypes
    condt = np.ascontiguousarray(
        cond_c.reshape(4, 4 * J, 10).transpose(0, 2, 1).reshape(40, 4 * J)
        .astype(ml_dtypes.bfloat16))
    # wixy[8g+d, comp*J+j] = wi[g*J+j, comp]  (d broadcast)
    w = wi_c.reshape(G, J, 2).transpose(0, 2, 1).reshape(G, 1, 2 * J)
    wixy = np.ascontiguousarray(np.broadcast_to(w, (G, 8, 2 * J))
                                .reshape(128, 2 * J))
    return {"condt": condt, "wixy": wixy}


def _unpack_ct(raw, J):
    """ct_raw[8g+d, j] -> cos_theta[g*J+j, d]."""
    return np.ascontiguousarray(
        raw.reshape(G, 8, J).transpose(0, 2, 1).reshape(G * J, 8))


def kernel(wi, cond, mlp0_ws, mlp0_bs, f0_ws, f0_bs, f1_w, f1_b):
    from concourse.bass_utils import run_bass_kernel_spmd

    wi = np.ascontiguousarray(np.asarray(wi, dtype=np.float32))
    cond = np.ascontiguousarray(np.asarray(cond, dtype=np.float32))
    B = wi.shape[0]
    Bc = B // NCORES
    J = Bc // G
    consts = _prep_consts(mlp0_ws, mlp0_bs, f0_ws, f0_bs, f1_w, f1_b)

    nc = _get_nc(J)
    in_maps = []
    for k in range(NCORES):
        m = _prep_core_inputs(wi[k * Bc:(k + 1) * Bc],
                              cond[k * Bc:(k + 1) * Bc], J)
        m.update(consts)
        in_maps.append(m)
    global _LAST_RESULT
    res = run_bass_kernel_spmd(nc, in_maps, core_ids=list(range(NCORES)),
                               **_RUN_KWARGS)
    _LAST_RESULT = res
    cos_theta = np.concatenate([_unpack_ct(r["ct_raw"], J)
                                for r in res.results], axis=0)
    pdf = np.concatenate([r["pdf_out"] for r in res.results], axis=0)
    return cos_theta, pdf


# revision 36
# speedup vs baseline: 4.3859x; 1.7359x over previous
"""Trainium2 Bass kernel for nn_Base2D_80633716015315 (dense_mlp).

Math (per ray b, lobe d):
  feat = mlp0(cond)            # 10->32->32->32->32, relu between
  weight = relu(feat[:8]); v = feat[8:24] as [8,2]; ct = sigmoid(feat[24:32])
  v /= max(||v||, eps); wr = R(v) @ wi        # 2D rotation of wi by v
  x = [wr0, wr1, ct];  pe = [x, sin(2^k x), cos(2^k x)]  (27 feats)
  inter = relu(mlp(pe))        # 27->64->64->64->64->64, relu between + out
  lobes = softplus(inter . f1_w[d] + f1_b[d])
  pdf = sum_d lobes*weight
Returns (cos_theta [B,8], pdf [B]).

Sharding: pure data-parallel over B across 8 cores; weights replicated.

Per-core layout: Bc rows; partition index p = 8*g + d (g in 0..15 batch
groups, d in 0..7 lobes); free index j in 0..J-1; b = g*J + j.
Matmuls run feature-major (features on partitions) in float32r with
4-region tile_position packing; PSUM evacuation alternates ACT/DVE.
"""

import numpy as np

D = 8
G = 16
NCORES = 8

_BUILD_CACHE = {}
_RUN_KWARGS = {}
_LAST_RESULT = None


def _prep_consts(mlp0_ws, mlp0_bs, f0_ws, f0_bs, f1_w, f1_b):
    """Host-side constant tensors, all float32, laid out for the kernel."""
    f = lambda x: np.ascontiguousarray(np.asarray(x, dtype=np.float32))
    mlp0_ws = [f(w) for w in mlp0_ws]
    mlp0_bs = [f(b) for b in mlp0_bs]
    f0_ws = [f(w) for w in f0_ws]
    f0_bs = [f(b) for b in f0_bs]
    f1_w = f(f1_w)
    f1_b = f(f1_b)

    consts = {}
    # Permute the last mlp0 layer's output features so the blocks we need
    # later are contiguous: [v0(8), v1(8), wpre(8), ctpre(8)].
    perm = np.concatenate([
        np.arange(8, 24, 2),      # v0 = feat[8+2d]
        np.arange(9, 24, 2),      # v1 = feat[9+2d]
        np.arange(0, 8),          # weight pre-relu
        np.arange(24, 32),        # cos_theta pre-sigmoid
    ])
    w_last = mlp0_ws[3][perm, :]
    b_last = mlp0_bs[3][perm]

    # One packed lhsT bundle [9, 128, 128]:
    #  slots 0-3: mlp0 4-block-diagonal (layer0 K=40, else K=128)
    #  slots 4-8: f0 2-block-diagonal (layer0 K=54, else K=128)
    wall = np.zeros((9, 128, 128), np.float32)
    for l in range(4):
        w = w_last if l == 3 else mlp0_ws[l]
        K = w.shape[1]
        for q in range(4):
            wall[l, K * q:K * q + K, 32 * q:32 * q + 32] = w.T
    for l in range(5):
        w = f0_ws[l]
        K = w.shape[1]
        wall[4 + l, 0:K, 0:64] = w.T
        wall[4 + l, K:2 * K, 64:128] = w.T
    # layer-0 f0 block replicated at row 64 too (odd pairs read their PEX
    # block at base partition 64; matmul requires lhsT/rhs bases to match)
    w = f0_ws[0]
    wall[4, 64:64 + 27, 0:64] = w.T
    wall[4, 64 + 27:64 + 54, 64:128] = w.T
    import ml_dtypes
    consts["swall"] = wall.reshape(9 * 128, 128).astype(ml_dtypes.bfloat16)

    # Bias bundle [128, 12]: cols 0-3 mlp0 (x4 repl), 4-8 f0 (x2 repl),
    # 9 f1 (f1_b[p%8]), 10 eps, 11 pi/2.
    ball = np.zeros((128, 12), np.float32)
    for l in range(4):
        b = b_last if l == 3 else mlp0_bs[l]
        ball[:, l] = np.tile(b, 4)
    for l in range(5):
        ball[:, 4 + l] = np.tile(f0_bs[l], 2)
    ball[:, 9] = np.tile(f1_b, G)
    ball[:, 10] = 1e-24
    ball[:, 11] = np.pi / 2
    consts["ball"] = ball

    # f1: 16 lhsT variants [128, 32]; variant v serves pairs i with i%16==v.
    # Column c=2v+h gets f1w[(2i+h)%8] in row-block h; other columns zero, so
    # a 32-matmul accumulation group lands lobes for 16 pairs densely in one
    # [32, J] psum tile (row p%32).
    f1c = np.zeros((16 * 128, 32), np.float32)
    for v in range(16):
        for h in range(2):
            d = (2 * v + h) % 8
            f1c[128 * v + 64 * h:128 * v + 64 * h + 64, 2 * v + h] = f1_w.T[:, d]
    consts["sf1c"] = f1c.astype(ml_dtypes.bfloat16)

    # pdf reduction selector: out[m] = sum_d PDFT[8m+d]
    sel = np.zeros((128, 16), np.float32)
    for p in range(128):
        sel[p, p // 8] = 1.0
    consts["sel"] = sel
    return consts


def _build(J, debug=False):
    """Build the single-core Bass program (run SPMD across all cores)."""
    from contextlib import ExitStack
    import concourse.bass as bass
    import concourse.mybir as mybir
    import concourse.tile as tile
    from concourse import bacc

    f32 = mybir.dt.float32
    f32r = mybir.dt.float32r
    bf16 = mybir.dt.bfloat16
    AF = mybir.ActivationFunctionType
    ALU = mybir.AluOpType

    Bc = G * J
    N0 = J // 2
    NCH0 = Bc // N0          # 32 mlp0 chunks
    NPAIR = (G * D) // 2     # 64 stage-C pair iterations

    nc = bacc.Bacc("TRN2", target_bir_lowering=False, debug=debug)

    # ---- DRAM I/O (cond/wi are pre-laid-out on the host) ----
    condt_d = nc.dram_tensor("condt", [40, 4 * J], bf16, kind="ExternalInput")
    wixy_d = nc.dram_tensor("wixy", [128, 2 * J], f32, kind="ExternalInput")
    cdram = {}
    cshapes = {}
    for name, shape in [
        ("swall", (9 * 128, 128)),
        ("ball", (128, 12)),
        ("sf1c", (16 * 128, 32)),
        ("sel", (128, 16)),
    ]:
        wdt = {"swall": bf16, "sf1c": bf16, "sel": f32r, "ball": f32}[name]
        cdram[name] = nc.dram_tensor(name, list(shape), wdt, kind="ExternalInput")
        cshapes[name] = shape
    ct_out_d = nc.dram_tensor("ct_raw", [128, J], f32, kind="ExternalOutput")
    pdf_out_d = nc.dram_tensor("pdf_out", [Bc], f32, kind="ExternalOutput")
    featd = nc.dram_tensor("featd", [128, 4 * J], f32)   # HBM bounce

    # ACT/DVE duty split for psum evacuations (ACT also does sin/sigmoid etc)
    ACT_FRAC = 0.55
    evac_state = {"n": 0, "acts": 0}

    def pick_act():
        evac_state["n"] += 1
        want = int(evac_state["n"] * ACT_FRAC)
        if evac_state["acts"] < want:
            evac_state["acts"] += 1
            return True
        return False

    def evac(out_ap, in_ap, bias_ap, relu):
        if relu:
            if pick_act():
                nc.scalar.activation(out_ap, in_ap, AF.Relu, bias=bias_ap)
            else:
                nc.vector.tensor_scalar(out_ap, in_ap, bias_ap, 0.0,
                                        ALU.add, ALU.max)
        else:
            if pick_act():
                nc.scalar.activation(out_ap, in_ap, AF.Identity, bias=bias_ap)
            else:
                nc.vector.tensor_scalar(out_ap, in_ap, bias_ap, None, ALU.add)

    with tile.TileContext(nc) as tc, ExitStack() as ctx:
        pc = ctx.enter_context(tc.tile_pool(name="consts", bufs=1))
        pkeep = ctx.enter_context(tc.tile_pool(name="keep", bufs=1))
        pfeat = ctx.enter_context(tc.tile_pool(name="feat", bufs=1))
        pqall = ctx.enter_context(tc.tile_pool(name="qall", bufs=1))

        WALL = pc.tile([128, 9 * 128], bf16, tag="wall", name="wall")
        nc.sync.dma_start(
            out=WALL[:],
            in_=bass.AP(tensor=cdram["swall"].ap().tensor, offset=0,
                        ap=[[128, 128], [128 * 128, 9], [1, 128]]))
        BALL = pc.tile([128, 12], f32, tag="ball", name="ball")
        nc.sync.dma_start(out=BALL[:], in_=cdram["ball"].ap())
        F1CALL = pc.tile([128, 16 * 32], bf16, tag="f1call", name="f1call")
        nc.sync.dma_start(
            out=F1CALL[:],
            in_=bass.AP(tensor=cdram["sf1c"].ap().tensor, offset=0,
                        ap=[[32, 128], [128 * 32, 16], [1, 32]]))
        SEL = pc.tile([128, 16], f32r, tag="sel", name="sel")
        nc.sync.dma_start(out=SEL[:], in_=cdram["sel"].ap())

        def wmat(slot, K):
            return WALL[0:K, 128 * slot:128 * slot + 128]

        def bias(col):
            return BALL[:, col:col + 1]

        WPRE = pkeep.tile([128, J], f32, tag="wpre")    # -> WEIGHT (in place)
        CTOUT = pkeep.tile([128, J], f32, tag="ctout")  # fp32 cos_theta out
        LOB = pkeep.tile([128, J], f32r, tag="lob")      # lobespre -> lobes -> pdft
        PDFSB = pkeep.tile([16, J], f32, tag="pdfsb")

        FEAT = pfeat.tile([128, 4 * J], f32, tag="feat", name="feat")
        QALL = pqall.tile([128, 27 * J], bf16, tag="qall")

        pps = ctx.enter_context(tc.tile_pool(name="ps", bufs=3, space="PSUM"))
        ppsf1 = ctx.enter_context(tc.tile_pool(name="psf1", bufs=1, space="PSUM"))

        # ================= Stage A: mlp0 (4-block-diagonal) =================
        pm = ctx.enter_context(tc.tile_pool(name="mlp0io", bufs=1))
        ph0 = ctx.enter_context(tc.tile_pool(name="h0", bufs=2))
        if True:
            CONDT = pm.tile([40, 8 * N0], bf16, tag="condt", name="condt")
            nc.sync.dma_start(out=CONDT[:], in_=condt_d.ap())

            KS0 = [40, 128, 128, 128]
            for jj in range(2):                 # two chains of jp pairs
                h_prev = [None, None]
                for l in range(4):
                    K = KS0[l]
                    pss = []
                    for c2 in range(2):
                        jp = 2 * jj + c2
                        ps = pps.tile([128, J], f32, tag="ps", name="ps")
                        for c in range(2):
                            if l == 0:
                                rhs = CONDT[:, (2 * jp + c) * N0:
                                            (2 * jp + c + 1) * N0]
                            else:
                                rhs = h_prev[c2][:, c * N0:(c + 1) * N0]
                            nc.tensor.matmul(
                                ps[:, c * N0:(c + 1) * N0],
                                wmat(l, K),
                                rhs,
                            )
                        pss.append(ps)
                    for c2 in range(2):
                        jp = 2 * jj + c2
                        if l < 3:
                            h = ph0.tile([128, J], bf16, tag="h0", name="h0t")
                            evac(h[:], pss[c2][:], bias(l), relu=True)
                            h_prev[c2] = h
                        else:
                            evac(FEAT[:, jp * J:(jp + 1) * J], pss[c2][:],
                                 bias(l), relu=False)

            pb = ctx.enter_context(tc.tile_pool(name="bvars", bufs=1))
            V0 = pb.tile([128, J], f32, tag="v0", name="v0")
            V1 = pb.tile([128, J], f32, tag="v1", name="v1")
            CTP = pb.tile([128, J], f32, tag="ctp", name="ctp")
            WIXY = pb.tile([128, 2 * J], f32, tag="wixy", name="wixy")
            T0 = pb.tile([128, J], f32, tag="t0", name="t0")
            T1 = pb.tile([128, J], f32, tag="t1", name="t1")

            nc.sync.dma_start(out=featd.ap(), in_=FEAT[:])
            for qty, dst in ((0, V0), (1, V1), (2, WPRE), (3, CTP)):
                for b_blk in range(4):
                    src = bass.AP(tensor=featd.ap().tensor,
                                  offset=(32 * b_blk + 8 * qty) * (4 * J),
                                  ap=[[J, 4], [4 * J, 8], [1, J]])
                    eng = nc.sync if qty % 2 == 0 else nc.scalar
                    eng.dma_start(out=dst[32 * b_blk:32 * b_blk + 32, :],
                                  in_=src)

            # wi: host-prepped broadcast layout, one DMA
            nc.sync.dma_start(out=WIXY[:], in_=wixy_d.ap())
            WIX = WIXY[:, 0:J]
            WIY = WIXY[:, J:2 * J]

            wr0 = QALL[:, 0:J]
            wr1 = QALL[:, J:2 * J]
            ct = QALL[:, 2 * J:3 * J]

            # rinv = exp(-0.5*ln(v0^2+v1^2+eps))
            nc.vector.tensor_tensor(T0[:], V0[:], V0[:], ALU.mult)
            nc.vector.tensor_tensor(T1[:], V1[:], V1[:], ALU.mult)
            nc.vector.tensor_tensor(T0[:], T0[:], T1[:], ALU.add)
            nc.scalar.activation(T1[:], T0[:], AF.Ln, bias=bias(10))
            nc.scalar.activation(T0[:], T1[:], AF.Exp, scale=-0.5)
            # normalize v
            nc.vector.tensor_tensor(V0[:], V0[:], T0[:], ALU.mult)
            nc.vector.tensor_tensor(V1[:], V1[:], T0[:], ALU.mult)
            # wr = R(v) wi
            nc.vector.tensor_tensor(T0[:], V0[:], WIX, ALU.mult)
            nc.vector.tensor_tensor(T1[:], V1[:], WIY, ALU.mult)
            nc.vector.tensor_tensor(wr0, T0[:], T1[:], ALU.subtract)
            nc.vector.tensor_tensor(T0[:], V1[:], WIX, ALU.mult)
            nc.vector.tensor_tensor(T1[:], V0[:], WIY, ALU.mult)
            nc.vector.tensor_tensor(wr1, T0[:], T1[:], ALU.add)
            # ct = sigmoid(ctpre): bf16 copy feeds the PE features, fp32
            # copy is the returned output
            nc.scalar.activation(ct, CTP[:], AF.Sigmoid)
            nc.scalar.activation(CTOUT[:], CTP[:], AF.Sigmoid)
            # weight = relu(wpre), in place
            nc.vector.tensor_scalar_max(WPRE[:], WPRE[:], 0.0)
            # positional encoding. ACT Sin only accepts [-pi, pi]; k=0
            # inputs are in range (|wr|<=1, ct in [0,1], +pi/2 <= 2.58), so
            # compute sin/cos at k=0 on ACT and k=1..3 by double-angle
            # recurrences (sin2x=2 s c, cos2x=1-2 s^2) on DVE/GPSIMD.
            T2 = pb.tile([128, J], f32, tag="t2", name="t2")
            T3 = pb.tile([128, J], f32, tag="t3", name="t3")

            def sinf(k, ci):
                f = 3 + 3 * k + ci
                return QALL[:, f * J:(f + 1) * J]

            def cosf(k, ci):
                f = 15 + 3 * k + ci
                return QALL[:, f * J:(f + 1) * J]

            for ci, comp in enumerate((wr0, wr1, ct)):
                nc.scalar.activation(sinf(0, ci), comp, AF.Sin)
                nc.scalar.activation(cosf(0, ci), comp, AF.Sin, bias=bias(11))
            for ci in range(3):
                eng = nc.gpsimd if ci == 1 else nc.vector
                ta, tb = (T2, T3) if ci == 1 else (T0, T1)
                for k in range(3):
                    s_k, c_k = sinf(k, ci), cosf(k, ci)
                    eng.tensor_tensor(ta[:], s_k, c_k, ALU.mult)
                    nc.vector.tensor_scalar(sinf(k + 1, ci), ta[:], 2.0,
                                            None, ALU.mult)
                    eng.tensor_tensor(tb[:], s_k, s_k, ALU.mult)
                    nc.vector.tensor_scalar(cosf(k + 1, ci), tb[:], -2.0,
                                            1.0, ALU.mult, ALU.add)

            # cos_theta: contiguous raw dump; host reshapes to [Bc, 8]
            nc.sync.dma_start(out=ct_out_d.ap(), in_=CTOUT[:])

        # ================= Stage C: f0 MLP + f1 =================
        # Two pair-chains processed in lockstep per PEX tile so the PE
        # alternates between independent dependency chains (keeps it dense
        # and HAM-warm): per layer emit MMs for pair 2e then 2e+1, then both
        # evacuations.
        KSF = [54, 128, 128, 128, 128]
        with tc.tile_pool(name="pex", bufs=4) as ppex, \
             tc.tile_pool(name="hc", bufs=8) as phc, \
             tc.tile_pool(name="scrap", bufs=2) as pscrap:
            psf1 = None
            qpitch = QALL.ap[0][0]
            for e in range(NPAIR // 2):
                pex2 = ppex.tile([128, J], bf16, tag="pex", name="pex")
                for half in range(2):
                    src = bass.AP(
                        tensor=QALL.tensor,
                        offset=QALL.offset + (4 * e + 2 * half) * qpitch,
                        ap=[[qpitch, 2], [J, 27], [1, J]])
                    eng = nc.sync if (e + half) % 2 == 0 else nc.scalar
                    eng.dma_start(out=pex2[64 * half:64 * half + 54, :], in_=src)

                h_prev = [None, None]
                for l in range(5):
                    K = KSF[l]
                    pss = []
                    for c in range(2):
                        ps = pps.tile([128, J], f32, tag="ps", name="ps")
                        for j in range(2):
                            rhs = (pex2[64 * c:64 * c + K, j * N0:(j + 1) * N0]
                                   if l == 0 else
                                   h_prev[c][0:K, j * N0:(j + 1) * N0])
                            nc.tensor.matmul(
                                ps[:, j * N0:(j + 1) * N0],
                                WALL[64 * c if l == 0 else 0:
                                     (64 * c if l == 0 else 0) + K,
                                     128 * (4 + l):128 * (4 + l) + 128],
                                rhs,
                            )
                        pss.append(ps)
                    for c in range(2):
                        hn = phc.tile([128, J], bf16, tag="h", name="hct")
                        evac(hn[:], pss[c][:], bias(4 + l), relu=True)
                        h_prev[c] = hn

                # f1: 16-pair accumulation group -> lobes land densely at
                # psum row p%32 (see _prep_consts for the F1C layout).
                if e % 8 == 0:
                    psf1 = ppsf1.tile([32, J], f32, tag="psf1", name="psf1")
                for c in range(2):
                    i = 2 * e + c
                    v = i % 16
                    for j in range(2):
                        nc.tensor.matmul(
                            psf1[:, j * N0:(j + 1) * N0],
                            F1CALL[:, 32 * v:32 * v + 32],
                            h_prev[c][:, j * N0:(j + 1) * N0],
                            start=(v == 0),
                            stop=(v == 15),
                            skip_group_check=True,
                        )
                if e % 8 == 7:
                    eg = e // 8
                    scrap = pscrap.tile([32, J], f32r, tag="scrap", name="scrap")
                    if pick_act():
                        nc.scalar.activation(scrap[:], psf1[:], AF.Identity)
                    else:
                        nc.vector.tensor_copy(scrap[:], psf1[:])
                    nc.scalar.dma_start(out=LOB[32 * eg:32 * eg + 32, :],
                                        in_=scrap[:])

        # ================= Final: softplus + pdf =================
        # X = lobespre + f1_b; softplus(X) = max(X,0) + ln(1 + exp(-|X|))
        # (no Softplus activation table on this build; FEAT tiles are dead
        # here and reused as scratch)
        S0, S1 = FEAT[:, 0:J], FEAT[:, J:2 * J]
        nc.vector.tensor_scalar(LOB[:], LOB[:], bias(9), None, ALU.add)
        nc.vector.tensor_scalar(S0[:], LOB[:], -1.0, None, ALU.mult)
        nc.vector.tensor_tensor(S0[:], LOB[:], S0[:], ALU.min)
        nc.scalar.activation(S0[:], S0[:], AF.Exp)
        nc.scalar.activation(S0[:], S0[:], AF.Ln, bias=1.0)
        nc.vector.tensor_scalar_max(S1[:], LOB[:], 0.0)
        nc.vector.tensor_tensor(LOB[:], S0[:], S1[:], ALU.add)
        nc.vector.tensor_tensor(LOB[:], LOB[:], WPRE[:], ALU.mult)
        pspdf = pps.tile([128, J], f32, tag="ps", name="pspdf")
        for j in range(2):
            nc.tensor.matmul(
                pspdf[0:16, j * N0:(j + 1) * N0],
                SEL[:, :],
                LOB[:, j * N0:(j + 1) * N0],
            )
        nc.scalar.activation(PDFSB[:], pspdf[0:16, :], AF.Identity)
        pdf_r = pdf_out_d.ap().rearrange("(g j) -> g j", g=G)
        nc.sync.dma_start(out=pdf_r, in_=PDFSB[:])

    nc.compile()
    return nc


def _get_nc(J, debug=False):
    key = (J, debug)
    if key not in _BUILD_CACHE:
        _BUILD_CACHE[key] = _build(J, debug=debug)
    return _BUILD_CACHE[key]


def _prep_core_inputs(wi_c, cond_c, J):
    """Host-side layout staging for one core's wi/cond slices."""
    # condt[10q+f, b'] = cond[q*4J + b', f]
    import ml_dtypes
    condt = np.ascontiguousarray(
        cond_c.reshape(4, 4 * J, 10).transpose(0, 2, 1).reshape(40, 4 * J)
        .astype(ml_dtypes.bfloat16))
    # wixy[8g+d, comp*J+j] = wi[g*J+j, comp]  (d broadcast)
    w = wi_c.reshape(G, J, 2).transpose(0, 2, 1).reshape(G, 1, 2 * J)
    wixy = np.ascontiguousarray(np.broadcast_to(w, (G, 8, 2 * J))
                                .reshape(128, 2 * J))
    return {"condt": condt, "wixy": wixy}


def _unpack_ct(raw, J):
    """ct_raw[8g+d, j] -> cos_theta[g*J+j, d]."""
    return np.ascontiguousarray(
        raw.reshape(G, 8, J).transpose(0, 2, 1).reshape(G * J, 8))


def kernel(wi, cond, mlp0_ws, mlp0_bs, f0_ws, f0_bs, f1_w, f1_b):
    from concourse.bass_utils import run_bass_kernel_spmd

    wi = np.ascontiguousarray(np.asarray(wi, dtype=np.float32))
    cond = np.ascontiguousarray(np.asarray(cond, dtype=np.float32))
    B = wi.shape[0]
    Bc = B // NCORES
    J = Bc // G
    consts = _prep_consts(mlp0_ws, mlp0_bs, f0_ws, f0_bs, f1_w, f1_b)

    nc = _get_nc(J)
    in_maps = []
    for k in range(NCORES):
        m = _prep_core_inputs(wi[k * Bc:(k + 1) * Bc],
                              cond[k * Bc:(k + 1) * Bc], J)
        m.update(consts)
        in_maps.append(m)
    global _LAST_RESULT
    res = run_bass_kernel_spmd(nc, in_maps, core_ids=list(range(NCORES)),
                               **_RUN_KWARGS)
    _LAST_RESULT = res
    cos_theta = np.concatenate([_unpack_ct(r["ct_raw"], J)
                                for r in res.results], axis=0)
    pdf = np.concatenate([r["pdf_out"] for r in res.results], axis=0)
    return cos_theta, pdf
